# Initial kernel scaffold
#
"""BERT self-attention block (QKV -> attention -> dense -> residual+LN) on 8 trn2 NeuronCores.

Sharding: data-parallel over batch (2) x tensor-parallel over heads (4 heads/core).
Per-core dense partials are summed with a chunked bf16 ReduceScatter over each
batch group ([[0,1,2,3],[4,5,6,7]]); each core finishes residual+LayerNorm on its
own token shard and the host reassembles the full [2, 2048, 1024] output.

Perf structure (measured ~247us vs 305us staged baseline):
- softmax exp is split between the scalar engine (ACT spline exp) and a
  custom vector-engine op (quadratic poly + 4 squarings ~= exp(x/8)),
  alternating engines within each kc pair so both probs of a pair finish
  together;
- probs are written as fp8e4 in kc pairs and each pair is one DoubleRow
  ctx matmul (2 fp8 weights/cell, K=256) -- halves the ctx PE slots;
- the scores PSUM pool is triple-buffered (with dense/V tiles rotating
  through the same pool) so the PE streams without >3.4us idle windows
  that would re-throttle the HAM clock gate to 4/8;
- QKV projections run in fp8 DoubleRow (weights prescaled x32 on the host,
  the resulting x1024 score scale folded into the exp constants, denominator
  'ones' set to 32.0 so the softmax ratio is unscaled), c-outer so compute
  starts while input DMAs are in flight; qkT bias-evacuation runs on the
  then-idle scalar engine via Act.Identity's per-partition bias.
"""

import sys

for _p in ("/opt/trn_rl_repo",):
    if _p not in sys.path:
        sys.path.insert(0, _p)

import numpy as np
import ml_dtypes

import concourse.bass as bass
import concourse.mybir as mybir
import concourse.tile as tile
from concourse import bacc
from concourse.bass_utils import run_bass_kernel_spmd

BF16 = ml_dtypes.bfloat16
FP8 = ml_dtypes.float8_e4m3

HIDDEN = 1024
HEADS = 16
HD = 64  # head dim
B = 2
S = 2048
LN_EPS = 1e-5

N_CORES = 8
TP = 4  # tensor-parallel ranks per batch group
LHEADS = HEADS // TP  # 4 local heads
PAIRS = LHEADS // 2  # 2 head pairs
NCD = HIDDEN // 128  # 8 contraction chunks
NTOK = S // 128  # 16 token chunks
NQT = 4  # attention q-tiles (512 q each)
QT = S // NQT  # 512
REPLICA_GROUPS = [[0, 1, 2, 3], [4, 5, 6, 7]]
# ReduceScatter chunk boundaries in 128-token units
RS_CHUNKS = [(0, 4), (4, 8), (8, 11), (11, 14), (14, 16)]
NCHUNK = len(RS_CHUNKS)
# per-rank rows per chunk (chunk token count / 4 ranks)
RS_SZ = [(hi - lo) * 32 for lo, hi in RS_CHUNKS]
# padded layout: chunk g's rows live at [g*128, g*128+sz) in hs_res / out
PAD_ROWS = NCHUNK * 128

# which kc chunks the vector engine handles for exp (rest go to ACT)
DVE_KC = frozenset((1, 5, 7, 11, 13))

dt = mybir.dt
Alu = mybir.AluOpType
Act = mybir.ActivationFunctionType

# ---------------- custom DVE op: poly exp ----------------
# out = (imm2 + x*(s0 + x*s1))^16  ~=  exp(x/8) for x in +-28 (raw q.k scores)
# (quadratic fit of exp(t) on t = x/128 in +-0.225, then 4 squarings)
_CQ = (1.00004518, 1.00351622, 0.49634025)
EXP_S0 = float(_CQ[1] / 128 / 1024)
EXP_S1 = float(_CQ[2] / (128 * 128) / (1024 * 1024))
EXP_IMM2 = float(_CQ[0])


def _register_exp_op():
    from concourse import dve_ops as DO
    from concourse.dve_spec import Spec, Src0, C0, C1, C2, lower
    from concourse.dve_spec import _has_src1 as has_src1
    from concourse.dve_uop import DveOpSpec

    name = "EXP_Q4_ANT"
    for o in DO.OPS:
        if o.name == name:
            return o
    a1 = Src0 * C1 + C0
    a2 = Src0 * a1 + C2
    p2 = a2 * a2
    p4 = p2 * p2
    p8 = p4 * p4
    body = p8 * p8

    def _ref(in0, in1, s0, s1, imm2):
        p = imm2 + in0 * (s0 + in0 * s1)
        for _ in range(4):
            p = p * p
        return p

    spec = Spec(body=body, reference=_ref)
    row = DO._CUSTOM_DVE_ROW_BASE + len(DO.OPS)
    DO._SUB_OPCODE_FOR_NAME[name] = row
    shas = {}
    for ver in ("v3", "v4"):
        uops = lower(spec, ver=ver)
        shas[ver] = DveOpSpec(
            name=name, opcode=row, uops=uops, rd1_en=has_src1(spec)
        ).sha(ver)
    op = DO.DveOp(name, spec, subdim=False, uops_sha=shas)
    DO.OPS.append(op)
    DO.CUSTOM_DVE_SPECS[name] = spec
    return op


EXP_OP = _register_exp_op()


def _build_program():
    nc = bacc.Bacc(
        "TRN2", target_bir_lowering=False, debug=False, num_devices=N_CORES
    )

    # Route Exp and Ln to the one table set that holds both, so the kernel
    # never reloads ACT tables (set ids are positional; only values change).
    from concourse import hw_specs

    for name, funcs in hw_specs.get_activation_tables(nc.m.arch).items():
        if name != "natural_log_exp_and_others":
            funcs.discard(Act.Exp)
            funcs.discard(Act.Ln)

    # ---------------- DRAM I/O ----------------
    hsT = nc.dram_tensor("hsT", [HIDDEN, S], dt.float8e4, kind="ExternalInput")
    wqk = nc.dram_tensor("wqk", [HIDDEN, 512], dt.float8e4, kind="ExternalInput")
    wv = nc.dram_tensor("wv", [HIDDEN, 256], dt.float8e4, kind="ExternalInput")
    wd = nc.dram_tensor("wd", [256, HIDDEN], dt.bfloat16, kind="ExternalInput")
    bqk = nc.dram_tensor("bqk", [512, 1], dt.float32, kind="ExternalInput")
    hs_res = nc.dram_tensor(
        "hs_res", [PAD_ROWS, HIDDEN], dt.bfloat16, kind="ExternalInput"
    )
    out = nc.dram_tensor("out", [PAD_ROWS, HIDDEN], dt.float32, kind="ExternalOutput")

    # internal DRAM for the collective (cannot use I/O tensors)
    cc_in = [
        nc.dram_tensor(f"cc_in{g}", [(hi - lo) * 128, HIDDEN], dt.bfloat16)
        for g, (lo, hi) in enumerate(RS_CHUNKS)
    ]
    cc_out = [
        nc.dram_tensor(f"cc_out{g}", [RS_SZ[g], HIDDEN], dt.bfloat16)
        for g in range(NCHUNK)
    ]

    with tile.TileContext(nc) as tc:
        with (
            tc.tile_pool(name="persist", bufs=1) as persist,
            tc.tile_pool(name="pT_pool", bufs=6) as pT_pool,
            tc.tile_pool(name="work", bufs=3) as work,
            tc.tile_pool(name="ln", bufs=2) as lnp,
        ):
            # ---------------- persistent SBUF loads ----------------
            zero_sb = persist.tile([128, 1], dt.float32, name="zero_sb")
            nc.vector.memset(zero_sb, 0.0)
            nc.const_aps.aps[(dt.float32, 0.0)] = zero_sb
            eps_sb = persist.tile([128, 1], dt.float32, name="eps_sb")
            nc.vector.memset(eps_sb, LN_EPS)
            # input DMAs: interleaved so the c-outer QK matmuls can start
            # after the first hsT/wqk chunk pair lands (the sync queue
            # serializes at ~0.6us per dma_start dispatch, so keep them few)
            hsT_all = persist.tile([128, NCD, S], dt.float8e4, name="hsT_all")
            hsT_r = hsT[:, :].rearrange("(c p) t -> p c t", p=128)
            wqk_all = persist.tile([128, NCD, 512], dt.float8e4, name="wqk_all")
            wqk_r = wqk[:, :].rearrange("(c p) n -> p c n", p=128)
            nc.sync.dma_start(out=wqk_all[:, 0:8, :], in_=wqk_r[:, 0:8, :])
            nc.sync.dma_start(out=hsT_all[:, 0:2, :], in_=hsT_r[:, 0:2, :])
            nc.sync.dma_start(out=hsT_all[:, 2:4, :], in_=hsT_r[:, 2:4, :])
            nc.sync.dma_start(out=hsT_all[:, 4:6, :], in_=hsT_r[:, 4:6, :])
            nc.sync.dma_start(out=hsT_all[:, 6:8, :], in_=hsT_r[:, 6:8, :])
            wv_all = persist.tile([128, NCD, 256], dt.float8e4, name="wv_all")
            nc.sync.dma_start(
                out=wv_all, in_=wv[:, :].rearrange("(c p) n -> p c n", p=128)
            )
            wd_all = persist.tile([128, 2, HIDDEN], dt.bfloat16, name="wd_all")
            nc.sync.dma_start(
                out=wd_all, in_=wd[:, :].rearrange("(c p) n -> p c n", p=128)
            )
            bqk_all = persist.tile([128, 4], dt.float32, name="bqk_all")
            nc.sync.dma_start(
                out=bqk_all, in_=bqk[:, :].rearrange("(m p) o -> p (m o)", p=128)
            )
            res_all = persist.tile([128, NCHUNK, HIDDEN], dt.bfloat16, name="res_all")
            nc.sync.dma_start(
                out=res_all,
                in_=hs_res[:, :].rearrange("(g p) n -> p g n", p=128),
            )
            hsT_sb = [hsT_all[:, c, :] for c in range(NCD)]
            wqk_sb = [wqk_all[:, c, :] for c in range(NCD)]
            wv_sb = [wv_all[:, c, :] for c in range(NCD)]
            wd_sb = [wd_all[:, c, :] for c in range(2)]
            bqk_sb = [bqk_all[:, m : m + 1] for m in range(4)]

            # qkT m-chunk layout: 0=K pair0, 1=Q pair0, 2=K pair1, 3=Q pair1
            # (partitions 0:64 = even head of the pair, 64:128 = odd head)
            qkT_sb = [
                persist.tile([128, S], dt.bfloat16, name=f"qkT{m}") for m in range(4)
            ]
            # V tiles (fp8, DoubleRow pairs): tile t2 slot s covers token
            # chunk 2*t2+s as 4 head-groups of [V_h(64) | ones(64)]
            v2_sb = [
                persist.tile([128, 2, 512], dt.float8e4, name=f"v{t2}")
                for t2 in range(NTOK // 2)
            ]
            # the denominator 'ones' (=32, matching the x32 wv prescale) never
            # change: write them all here while the vector engine is idle
            # instead of inside the qt0 attention weave
            for t2 in range(NTOK // 2):
                vt_all = v2_sb[t2].rearrange("p s (g c) -> p (s g) c", c=128)
                nc.vector.memset(vt_all[:, :, 64:128], 32.0)
            # ctx^T (normalized, bf16): chunk p holds heads 2p (parts 0:64), 2p+1
            ctxT_sb = [
                persist.tile([128, S], dt.bfloat16, name=f"ctxT{p}")
                for p in range(PAIRS)
            ]

            # ---------------- QK projection (c-outer, all 8 PSUM banks) -------
            # qk_ps region idx = m*4 + nh*2 + j accumulates over c; iteration c
            # only needs hsT chunk c + wqk chunk c, so compute starts while the
            # rest of the inputs are still in flight.
            with tc.tile_pool(name="psqk", bufs=1, space="PSUM") as psqk:
                for mp in range(2):  # m-pass: m in {2mp, 2mp+1} (8 banks each)
                    qk_ps = psqk.tile([128, 8, 512], dt.float32, name="qk_ps")
                    for cp in range(NCD // 2):
                        for dm in range(2):
                            m = 2 * mp + dm
                            for nh in range(2):
                                for j in range(2):
                                    nc.tensor.matmul(
                                        qk_ps[:, dm * 4 + nh * 2 + j, :],
                                        lhsT=wqk_all[
                                            :,
                                            2 * cp : 2 * cp + 2,
                                            m * 128 : (m + 1) * 128,
                                        ],
                                        rhs=hsT_all[
                                            :,
                                            2 * cp : 2 * cp + 2,
                                            nh * 1024
                                            + j * 512 : nh * 1024
                                            + (j + 1) * 512,
                                        ],
                                        start=(cp == 0),
                                        stop=(cp == NCD // 2 - 1),
                                        perf_mode=mybir.MatmulPerfMode.DoubleRow,
                                    )
                    for dm in range(2):
                        m = 2 * mp + dm
                        for nh in range(2):
                            nc.scalar.activation(
                                out=qkT_sb[m][:, nh * 1024 : (nh + 1) * 1024],
                                in_=qk_ps[
                                    :, dm * 4 + nh * 2 : dm * 4 + nh * 2 + 2, :
                                ],
                                func=Act.Identity,
                                bias=bqk_sb[m],
                            )

            # psqk released; attention pools take over PSUM
            with (
                tc.tile_pool(name="psmm", bufs=3, space="PSUM") as psmm,
                tc.tile_pool(name="psctx", bufs=1, space="PSUM") as psctx,
            ):
                # V[tc][:, l*128:l*128+64] = hs[tok_chunk] @ wv[:, l*64:...]
                # cols l*128+64 : (l+1)*128 are constant 1.0 (denominator trick)
                def emit_v_chunk(t):
                    ps = psmm.tile([128, 1024], dt.float32, name="ps_mm")
                    for cp in range(NCD // 2):
                        nc.tensor.matmul(
                            ps[:, 0:256],
                            lhsT=hsT_all[
                                :, 2 * cp : 2 * cp + 2, t * 128 : (t + 1) * 128
                            ],
                            rhs=wv_all[:, 2 * cp : 2 * cp + 2, :],
                            start=(cp == 0),
                            stop=(cp == NCD // 2 - 1),
                            perf_mode=mybir.MatmulPerfMode.DoubleRow,
                        )
                    vt = v2_sb[t // 2][:, t % 2, :].rearrange(
                        "p (g c) -> p g c", c=128
                    )
                    nc.vector.tensor_copy(
                        out=vt[:, :, 0:64],
                        in_=ps[:, 0:256].rearrange("p (g c) -> p g c", c=64),
                    )

                for t in range(4):
                    emit_v_chunk(t)

                # ---------------- phase 2: attention + dense + RS ----------------
                # q-tile-major so each RS chunk launches as early as possible --
                # the serialized CC-core queue is the kernel's tail constraint
                cc_insts = []
                dense_state = {"last_evac": None}

                def emit_dense_ti(ti_g):
                    tok = ti_g * 128
                    ps_d = psmm.tile([128, 1024], dt.float32, name="ps_mm")
                    for cc in range(2):
                        for j in range(2):
                            nc.tensor.matmul(
                                ps_d[:, j * 512 : (j + 1) * 512],
                                lhsT=ctxT_sb[cc][:, tok : tok + 128],
                                rhs=wd_sb[cc][:, j * 512 : (j + 1) * 512],
                                start=(cc == 0),
                                stop=(cc == 1),
                            )
                    dsb = work.tile([128, 1024], dt.bfloat16, name="dsb")
                    dense_state["last_evac"] = nc.vector.tensor_copy(
                        out=dsb, in_=ps_d
                    )
                    g = next(
                        i for i, (lo, hi) in enumerate(RS_CHUNKS) if lo <= ti_g < hi
                    )
                    lo = RS_CHUNKS[g][0]
                    nc.sync.dma_start(
                        out=cc_in[g][(ti_g - lo) * 128 : (ti_g - lo + 1) * 128, :],
                        in_=dsb,
                    )
                    if ti_g == RS_CHUNKS[g][1] - 1:
                        cc_insts.append(
                            nc.gpsimd.collective_compute(
                                "ReduceScatter",
                                Alu.add,
                                replica_groups=REPLICA_GROUPS,
                                ins=[cc_in[g][:, :].opt()],
                                outs=[cc_out[g][:, :].opt()],
                            )
                        )

                for qt in range(NQT):
                    for p in range(PAIRS):
                        km = 2 * p  # K m-chunk
                        qm = 2 * p + 1  # Q m-chunk
                        ctx_ps = [
                            psctx.tile([128, 512], dt.float32, name=f"ps_ctx{l}")
                            for l in range(2)
                        ]

                        def emit_scores(kc, km=km, qm=qm, qt=qt):
                            ps_s = psmm.tile([128, 1024], dt.float32, name="ps_mm")
                            # scores^T for both heads of the pair (concurrent row
                            # groups: even head rows 0:64, odd head rows 64:128)
                            for l in range(2):
                                nc.tensor.matmul(
                                    ps_s[:, l * 512 : (l + 1) * 512],
                                    lhsT=qkT_sb[km][
                                        l * 64 : (l + 1) * 64, kc * 128 : (kc + 1) * 128
                                    ],
                                    rhs=qkT_sb[qm][
                                        l * 64 : (l + 1) * 64, qt * 512 : (qt + 1) * 512
                                    ],
                                    start=True,
                                    stop=True,
                                    tile_position=(l * 64, 0),
                                )
                            return ps_s

                        # software pipeline: scores run one k-chunk ahead so the
                        # PE never sits in-order behind ctx(k)'s wait on exp(k).
                        # probs are written as fp8 in kc pairs; each pair is one
                        # DoubleRow ctx matmul (2 fp8 weights/cell, K=256).
                        ps_s = emit_scores(0)
                        pT2 = None
                        for kc in range(NTOK):
                            kc2, sl = kc // 2, kc % 2
                            if sl == 0:
                                pT2 = pT_pool.tile(
                                    [128, 2, 1024], dt.float8e4, name="pT2"
                                )
                            ps_s_next = emit_scores(kc + 1) if kc + 1 < NTOK else None
                            if kc in DVE_KC:
                                # vector-engine poly exp (frees the ACT engine)
                                nc.vector._custom_dve(
                                    EXP_OP,
                                    out=pT2[:, sl, :],
                                    in0=ps_s,
                                    s0=EXP_S0,
                                    s1=EXP_S1,
                                    imm2=EXP_IMM2,
                                )
                            else:
                                nc.scalar.activation(
                                    out=pT2[:, sl, :],
                                    in_=ps_s,
                                    func=Act.Exp,
                                    scale=0.125 / 1024,
                                )
                            ps_s = ps_s_next
                            # ctx^T (+ denominator rows 64:128): one DoubleRow
                            # matmul per kc pair per head, accumulated over kc2
                            if sl == 1:
                                for l in range(2):
                                    h = 2 * p + l
                                    nc.tensor.matmul(
                                        ctx_ps[l],
                                        lhsT=v2_sb[kc2][
                                            :, :, h * 128 : (h + 1) * 128
                                        ],
                                        rhs=pT2[:, :, l * 512 : (l + 1) * 512],
                                        start=(kc2 == 0),
                                        stop=(kc2 == NTOK // 2 - 1),
                                        perf_mode=mybir.MatmulPerfMode.DoubleRow,
                                    )
                            # first q-tile: produce the remaining V chunks just
                            # ahead of their use (ctx(kc) needs v_sb[kc]); later
                            # q-tiles: weave the previous q-tile's dense matmuls
                            # into the PE slack so the exp engines never stall on
                            # the in-order PE queue behind dense work
                            if p == 0 and qt == 0 and kc + 4 < NTOK:
                                emit_v_chunk(kc + 4)
                            # kc>=4 so the previous tile's ctxT normalize (DVE) has
                            # drained before the dense matmuls reach the PE queue
                            if (
                                p == 0
                                and qt >= 1
                                and kc >= 4
                                and (kc - 4) % 3 == 0
                                and (kc - 4) // 3 < 4
                            ):
                                emit_dense_ti((qt - 1) * 4 + (kc - 4) // 3)
                        # normalize: ctx[0:64] / den[64:128] -> ctxT (bf16);
                        # both heads' denominators share one reciprocal pass
                        den2 = work.tile([128, 512], dt.float32, name="den2")
                        for l in range(2):
                            nc.vector.tensor_copy(
                                out=den2[l * 64 : (l + 1) * 64, :],
                                in_=ctx_ps[l][64:128, :],
                            )
                        rec = work.tile([128, 512], dt.float32, name="rec")
                        nc.vector.reciprocal_approx_fast(out=rec, in_=den2)
                        for l in range(2):
                            nc.vector.tensor_tensor(
                                out=ctxT_sb[p][
                                    l * 64 : (l + 1) * 64, qt * 512 : (qt + 1) * 512
                                ],
                                in0=ctx_ps[l][0:64, :],
                                in1=rec[l * 64 : (l + 1) * 64, :],
                                op=Alu.mult,
                            )
                # last q-tile's dense has no following attention to hide in
                for ti in range(4):
                    emit_dense_ti(12 + ti)
                last_evac = dense_state["last_evac"]

                # ---------------- phase 3: residual + LayerNorm ----------------
                # Pin every LN chunk after the last dense evacuation so the
                # in-order engine queues never block on an RS mid-attention;
                # LN for the early chunks then fills the final RS waits.
                from concourse.bass import _add_dep_helper

                for g in range(NCHUNK):
                    sz = RS_SZ[g]
                    xb = lnp.tile([128, HIDDEN], dt.bfloat16, name="xb")
                    xb_dma = nc.sync.dma_start(out=xb[:sz, :], in_=cc_out[g][:, :])
                    _add_dep_helper(
                        xb_dma.ins,
                        last_evac.ins,
                        sync=True,
                        reason="LN after attention/dense (keep queues unblocked)",
                    )
                    x = lnp.tile([128, HIDDEN], dt.float32, name="x")
                    nc.vector.tensor_tensor(
                        out=x[:sz, :],
                        in0=xb[:sz, :],
                        in1=res_all[:sz, g, :],
                        op=Alu.add,
                    )
                    stats = lnp.tile([128, 2, 6], dt.float32, name="stats")
                    xv = x.rearrange("p (s f) -> p s f", f=512)
                    for i in range(2):
                        nc.vector.bn_stats(out=stats[:sz, i, :], in_=xv[:sz, i, :])
                    mv = lnp.tile([128, 2], dt.float32, name="mv")
                    nc.vector.bn_aggr(out=mv[:sz, :], in_=stats[:sz, :, :])
                    # rstd = exp(-0.5 * ln(var + eps)) -- stays in the exp/ln table set
                    lnv = lnp.tile([128, 1], dt.float32, name="lnv")
                    nc.scalar.activation(
                        out=lnv[:sz, :], in_=mv[:sz, 1:2], func=Act.Ln, bias=eps_sb[:sz, :]
                    )
                    rstd = lnp.tile([128, 1], dt.float32, name="rstd")
                    nc.scalar.activation(
                        out=rstd[:sz, :], in_=lnv[:sz, :], func=Act.Exp, scale=-0.5
                    )
                    y = lnp.tile([128, HIDDEN], dt.float32, name="y")
                    nc.vector.tensor_scalar(
                        out=y[:sz, :],
                        in0=x[:sz, :],
                        scalar1=mv[:sz, 0:1],
                        scalar2=rstd[:sz, :],
                        op0=Alu.subtract,
                        op1=Alu.mult,
                    )
                    nc.sync.dma_start(
                        out=out[g * 128 : g * 128 + sz, :], in_=y[:sz, :]
                    )

    nc.compile()
    return nc


_PROGRAM = None


def _get_program():
    global _PROGRAM
    if _PROGRAM is None:
        _PROGRAM = _build_program()
    return _PROGRAM


def _prep_core_inputs(hidden_states, w_qkv, b_qkv, w_dense, b_dense):
    """Build the 8 per-core input maps (numpy, host-side sharding)."""
    hs = np.asarray(hidden_states, dtype=np.float32)
    w_qkv = np.asarray(w_qkv, dtype=np.float32)
    b_qkv = np.asarray(b_qkv, dtype=np.float32)
    w_dense = np.asarray(w_dense, dtype=np.float32)
    b_dense = np.asarray(b_dense, dtype=np.float32)

    # v-channel bias folded into a host-side output bias:
    # b_out = b_dense + b_v_full @ w_dense   (b_v in ctx channel order)
    bv_full = np.empty((HIDDEN,), dtype=np.float64)
    for g in range(HEADS):
        bv_full[g * HD : (g + 1) * HD] = b_qkv[g * 192 + 128 : g * 192 + 192]
    # w_dense rows are already in (head, d) = g*64+d order, matching bv_full
    b_out = (
        b_dense.astype(np.float64)
        + bv_full @ w_dense.astype(np.float64)
    ).astype(np.float32)

    in_maps = []
    for r in range(N_CORES):
        b = r // TP
        tp = r % TP
        gheads = [4 * tp + l for l in range(LHEADS)]

        hsT_bf = np.ascontiguousarray(hs[b].T).astype(FP8)  # [1024, 2048]

        # wqk column order: per pair: K(even) K(odd) Q(even) Q(odd), 64 each
        wqk_cols = np.empty((HIDDEN, 512), dtype=np.float32)
        bqk_vec = np.empty((512,), dtype=np.float32)
        for p in range(PAIRS):
            for l in range(2):
                g = gheads[2 * p + l]
                kcol = slice(g * 192 + 64, g * 192 + 128)
                qcol = slice(g * 192, g * 192 + 64)
                base = p * 256
                wqk_cols[:, base + l * 64 : base + (l + 1) * 64] = w_qkv[:, kcol]
                wqk_cols[:, base + 128 + l * 64 : base + 128 + (l + 1) * 64] = w_qkv[
                    :, qcol
                ]
                bqk_vec[base + l * 64 : base + (l + 1) * 64] = b_qkv[kcol]
                bqk_vec[base + 128 + l * 64 : base + 128 + (l + 1) * 64] = b_qkv[qcol]

        wv_cols = np.empty((HIDDEN, 256), dtype=np.float32)
        for l, g in enumerate(gheads):
            wv_cols[:, l * 64 : (l + 1) * 64] = w_qkv[
                :, g * 192 + 128 : g * 192 + 192
            ]

        wd_rows = np.empty((256, HIDDEN), dtype=np.float32)
        for l, g in enumerate(gheads):
            wd_rows[l * 64 : (l + 1) * 64, :] = w_dense[g * 64 : (g + 1) * 64, :]

        # residual shard (+ folded output bias); padded layout: chunk g's
        # sz rows live at [g*128, g*128+sz), covering global tokens
        # lo*128 + tp*sz + [0, sz)
        res = np.zeros((PAD_ROWS, HIDDEN), dtype=np.float32)
        for g, (lo, hi) in enumerate(RS_CHUNKS):
            sz = RS_SZ[g]
            t0 = lo * 128 + tp * sz
            res[g * 128 : g * 128 + sz, :] = hs[b, t0 : t0 + sz, :] + b_out

        in_maps.append(
            {
                "hsT": hsT_bf,
                "wqk": (wqk_cols * 32).astype(FP8),
                "wv": (wv_cols * 32).astype(FP8),
                "wd": wd_rows.astype(BF16),
                "bqk": (bqk_vec * 32).reshape(512, 1),
                "hs_res": res.astype(BF16),
            }
        )
    return in_maps


def kernel(hidden_states, w_qkv, b_qkv, w_dense, b_dense, ln_gamma, ln_beta,
           _return_perf=False, **run_kwargs):
    ln_gamma = np.asarray(ln_gamma, dtype=np.float32)
    ln_beta = np.asarray(ln_beta, dtype=np.float32)
    gamma_one = np.allclose(ln_gamma, 1.0)
    beta_zero = np.allclose(ln_beta, 0.0)

    nc = _get_program()
    in_maps = _prep_core_inputs(hidden_states, w_qkv, b_qkv, w_dense, b_dense)
    res = run_bass_kernel_spmd(
        nc, in_maps, core_ids=list(range(N_CORES)), **run_kwargs
    )

    full = np.empty((B, S, HIDDEN), dtype=np.float32)
    for r in range(N_CORES):
        b = r // TP
        tp = r % TP
        o = res.results[r]["out"]
        for g, (lo, hi) in enumerate(RS_CHUNKS):
            sz = RS_SZ[g]
            t0 = lo * 128 + tp * sz
            full[b, t0 : t0 + sz, :] = o[g * 128 : g * 128 + sz, :]

    if not (gamma_one and beta_zero):
        # spec fills gamma=ones, beta=zeros; fall back on host if they differ
        full = full * ln_gamma[None, None, :] + ln_beta[None, None, :]

    if _return_perf:
        return full, res
    return full



# revision 28
# speedup vs baseline: 1.2068x; 1.2068x over previous
"""BERT self-attention block (QKV -> attention -> dense -> residual+LN) on 8 trn2 NeuronCores.

Sharding: tensor-parallel over heads across all 8 cores (2 heads/core), with BOTH
batch elements on every core (batch plays the old "head pair" role in the attention
weave). After attention, a per-q-tile 8-core AllToAll exchanges ctx^T (bf16,
256KB/chunk) so each core owns the full 1024 ctx channels for its 128-token shard
of the chunk; the core then computes the full dense projection + residual + LN
locally and the host reassembles the [2, 2048, 1024] output.

This replaces the old scheme (DP batch x TP=4 heads, dense partials summed with a
chunked ReduceScatter) whose serialized CC chain (4MB/core at ~25GB/s = ~156us)
dominated the tail: the A2A moves 4x fewer bytes and fires right after each
q-tile's attention instead of waiting for dense.

Perf structure (inherited from the tuned baseline):
- softmax exp is split between the scalar engine (ACT spline exp) and a
  custom vector-engine op (quadratic poly + 4 squarings ~= exp(x/8)),
  alternating engines within each kc pair so both probs of a pair finish
  together;
- probs are written as fp8e4 in kc pairs and each pair is one DoubleRow
  ctx matmul (2 fp8 weights/cell, K=256) -- halves the ctx PE slots;
- the scores PSUM pool is triple-buffered (with dense/V tiles rotating
  through the same pool) so the PE streams without >3.4us idle windows
  that would re-throttle the HAM clock gate to 4/8;
- QKV projections run in fp8 DoubleRow (weights prescaled x32 on the host,
  the resulting x1024 score scale folded into the exp constants, denominator
  'ones' set to 32.0 so the softmax ratio is unscaled), c-outer so compute
  starts while input DMAs are in flight; qkT bias-evacuation runs on the
  then-idle scalar engine via Act.Identity's per-partition bias.
"""

import sys

for _p in ("/opt/trn_rl_repo",):
    if _p not in sys.path:
        sys.path.insert(0, _p)

import numpy as np
import ml_dtypes

import concourse.bass as bass
import concourse.mybir as mybir
import concourse.tile as tile
from concourse import bacc
from concourse.bass_utils import run_bass_kernel_spmd

BF16 = ml_dtypes.bfloat16
FP8 = ml_dtypes.float8_e4m3

HIDDEN = 1024
HEADS = 16
HD = 64  # head dim
B = 2
S = 2048
LN_EPS = 1e-5

N_CORES = 8
LHEADS = 2  # heads per core
PAIRS = 2  # attention passes per q-tile: pair p = batch p (2 local heads each)
NCD = HIDDEN // 128  # 8 contraction chunks
NTOK = S // 128  # 16 token chunks (per batch)
NQT = 4  # attention q-tiles (512 q each)
QT = S // NQT  # 512
REPLICA_GROUPS = [[0, 1, 2, 3, 4, 5, 6, 7]]
# per-core output: for each q-tile, 64 tokens of each batch
# (rows qt*128 + b*64 + t  <->  full[b, qt*512 + rank*64 + t])
OUT_ROWS = NQT * 128  # 512

# which kc chunks the vector engine handles for exp (rest go to ACT)
DVE_KC = frozenset((1, 3, 5, 7, 11, 13))

dt = mybir.dt
Alu = mybir.AluOpType
Act = mybir.ActivationFunctionType

# ---------------- custom DVE op: poly exp ----------------
# out = (imm2 + x*(s0 + x*s1))^16  ~=  exp(x/8) for x in +-28 (raw q.k scores)
# (quadratic fit of exp(t) on t = x/128 in +-0.225, then 4 squarings)
_CQ = (1.00004518, 1.00351622, 0.49634025)
EXP_S0 = float(_CQ[1] / 128 / 1024)
EXP_S1 = float(_CQ[2] / (128 * 128) / (1024 * 1024))
EXP_IMM2 = float(_CQ[0])


def _register_exp_op():
    from concourse import dve_ops as DO
    from concourse.dve_spec import Spec, Src0, C0, C1, C2, lower
    from concourse.dve_spec import _has_src1 as has_src1
    from concourse.dve_uop import DveOpSpec

    name = "EXP_Q4_ANT"
    for o in DO.OPS:
        if o.name == name:
            return o
    a1 = Src0 * C1 + C0
    a2 = Src0 * a1 + C2
    p2 = a2 * a2
    p4 = p2 * p2
    p8 = p4 * p4
    body = p8 * p8

    def _ref(in0, in1, s0, s1, imm2):
        p = imm2 + in0 * (s0 + in0 * s1)
        for _ in range(4):
            p = p * p
        return p

    spec = Spec(body=body, reference=_ref)
    row = DO._CUSTOM_DVE_ROW_BASE + len(DO.OPS)
    DO._SUB_OPCODE_FOR_NAME[name] = row
    shas = {}
    for ver in ("v3", "v4"):
        uops = lower(spec, ver=ver)
        shas[ver] = DveOpSpec(
            name=name, opcode=row, uops=uops, rd1_en=has_src1(spec)
        ).sha(ver)
    op = DO.DveOp(name, spec, subdim=False, uops_sha=shas)
    DO.OPS.append(op)
    DO.CUSTOM_DVE_SPECS[name] = spec
    return op


EXP_OP = _register_exp_op()


def _build_program():
    nc = bacc.Bacc(
        "TRN2", target_bir_lowering=False, debug=False, num_devices=N_CORES
    )

    # Route Exp and Ln to the one table set that holds both, so the kernel
    # never reloads ACT tables (set ids are positional; only values change).
    from concourse import hw_specs

    for name, funcs in hw_specs.get_activation_tables(nc.m.arch).items():
        if name != "natural_log_exp_and_others":
            funcs.discard(Act.Exp)
            funcs.discard(Act.Ln)

    # ---------------- DRAM I/O ----------------
    # hsT: both batches, [1024, 4096] = [hid, b*2048 + t]
    hsT = nc.dram_tensor("hsT", [HIDDEN, B * S], dt.float8e4, kind="ExternalInput")
    # wqk: [1024, 256] = [K h0 | K h1 | Q h0 | Q h1] (x32 prescale)
    wqk = nc.dram_tensor("wqk", [HIDDEN, 256], dt.float8e4, kind="ExternalInput")
    # wv: [1024, 128] = [V h0 | V h1] (x32 prescale)
    wv = nc.dram_tensor("wv", [HIDDEN, 128], dt.float8e4, kind="ExternalInput")
    # wd: full dense weight [1024, 1024]
    wd = nc.dram_tensor("wd", [HIDDEN, HIDDEN], dt.float8e4, kind="ExternalInput")
    bqk = nc.dram_tensor("bqk", [256, 1], dt.float32, kind="ExternalInput")
    # residual (+ folded dense bias) for this core's token shard
    hs_res = nc.dram_tensor(
        "hs_res", [OUT_ROWS, HIDDEN], dt.bfloat16, kind="ExternalInput"
    )
    out = nc.dram_tensor("out", [OUT_ROWS, HIDDEN], dt.float32, kind="ExternalOutput")

    # internal DRAM for the collective (cannot use I/O tensors)
    # cc layout per qt: [8 peer blocks * 128 chan, 128] where block r =
    # my 128 channels for tokens qt*512 + r*64 (+64 of each batch:
    # cols 0:64 = batch0, 64:128 = batch1). NOTE: a [64, 2048] variant
    # (4KB rows) measured SLOWER -- the CC mesh parallelizes across rows,
    # so keep many rows.
    # qt0-2 exchange once per q-tile (fewer sync-queue wait points in the
    # steady state); the LAST q-tile splits per batch so the tail only
    # waits on a 64KB op and batch0's half hides under the p==1 pass
    cc_in = [
        [nc.dram_tensor(f"cc_in{q}", [N_CORES * 128, 128], dt.float8e4)]
        if q < NQT - 1
        else [
            nc.dram_tensor(f"cc_in{q}_{b}", [N_CORES * 128, 64], dt.float8e4)
            for b in range(2)
        ]
        for q in range(NQT)
    ]
    cc_out = [
        [nc.dram_tensor(f"cc_out{q}", [N_CORES * 128, 128], dt.float8e4)]
        if q < NQT - 1
        else [
            nc.dram_tensor(f"cc_out{q}_{b}", [N_CORES * 128, 64], dt.float8e4)
            for b in range(2)
        ]
        for q in range(NQT)
    ]
    # tiny dummy exchange fired at kernel start: absorbs the ~11.5us
    # first-collective trigger delay + CC DMA-ring spin-up so A2A(qt0)
    # runs at warm-stream speed
    cc_warm_in = nc.dram_tensor("cc_warm_in", [N_CORES, 128], dt.bfloat16)
    cc_warm_out = nc.dram_tensor("cc_warm_out", [N_CORES, 128], dt.bfloat16)

    with tile.TileContext(nc) as tc:
        with (
            tc.tile_pool(name="persist", bufs=1) as persist,
            tc.tile_pool(name="pT_pool", bufs=6) as pT_pool,
            tc.tile_pool(name="work", bufs=3) as work,
            tc.tile_pool(name="ln", bufs=2) as lnp,
        ):
            # ---------------- persistent SBUF loads ----------------
            zero_sb = persist.tile([128, 1], dt.float32, name="zero_sb")
            nc.vector.memset(zero_sb, 0.0)
            nc.const_aps.aps[(dt.float32, 0.0)] = zero_sb
            eps_sb = persist.tile([128, 1], dt.float32, name="eps_sb")
            nc.vector.memset(eps_sb, LN_EPS)
            # warm the CC stream before any data dep can delay the trigger
            nc.gpsimd.collective_compute(
                "AllToAll",
                Alu.bypass,
                replica_groups=REPLICA_GROUPS,
                ins=[cc_warm_in[:, :].opt()],
                outs=[cc_warm_out[:, :].opt()],
            )
            # input DMAs: interleaved so the c-outer QK matmuls can start
            # after the first hsT/wqk chunk pair lands (the sync queue
            # serializes at ~0.6us per dma_start dispatch, so keep them few).
            # wd/res (3MB) aren't consumed until ~120us in: dispatch them
            # last so they don't steal HBM bandwidth from the hsT stream.
            hsT_all = persist.tile([128, NCD, B * S], dt.float8e4, name="hsT_all")
            hsT_r = hsT[:, :].rearrange("(c p) t -> p c t", p=128)
            wqk_all = persist.tile([128, NCD, 256], dt.float8e4, name="wqk_all")
            wqk_r = wqk[:, :].rearrange("(c p) n -> p c n", p=128)
            nc.sync.dma_start(out=wqk_all[:, 0:8, :], in_=wqk_r[:, 0:8, :])
            nc.sync.dma_start(out=hsT_all[:, 0:2, :], in_=hsT_r[:, 0:2, :])
            nc.sync.dma_start(out=hsT_all[:, 2:4, :], in_=hsT_r[:, 2:4, :])
            nc.sync.dma_start(out=hsT_all[:, 4:6, :], in_=hsT_r[:, 4:6, :])
            nc.sync.dma_start(out=hsT_all[:, 6:8, :], in_=hsT_r[:, 6:8, :])
            bqk_all = persist.tile([128, 2], dt.float32, name="bqk_all")
            nc.sync.dma_start(
                out=bqk_all, in_=bqk[:, :].rearrange("(m p) o -> p (m o)", p=128)
            )
            wv_all = persist.tile([128, NCD, 128], dt.float8e4, name="wv_all")
            nc.sync.dma_start(
                out=wv_all, in_=wv[:, :].rearrange("(c p) n -> p c n", p=128)
            )
            wd_all = persist.tile([128, NCD, HIDDEN], dt.float8e4, name="wd_all")
            nc.sync.dma_start(
                out=wd_all, in_=wd[:, :].rearrange("(c p) n -> p c n", p=128)
            )
            res_all = persist.tile([128, NQT, HIDDEN], dt.bfloat16, name="res_all")
            nc.sync.dma_start(
                out=res_all,
                in_=hs_res[:, :].rearrange("(g p) n -> p g n", p=128),
            )
            bqk_sb = [bqk_all[:, m : m + 1] for m in range(2)]

            # qkT m-chunk layout: 0=K batch0, 1=Q batch0, 2=K batch1, 3=Q batch1
            # (partitions 0:64 = local head 0, 64:128 = local head 1)
            qkT_sb = [
                persist.tile([128, S], dt.bfloat16, name=f"qkT{m}") for m in range(4)
            ]
            # V tiles (fp8, DoubleRow pairs): tile t2 slot s covers token
            # chunk 2*t2+s as 4 groups (g = 2*batch + head) of [V_h(64) | ones(64)]
            v2_sb = [
                persist.tile([128, 2, 512], dt.float8e4, name=f"v{t2}")
                for t2 in range(NTOK // 2)
            ]
            # the denominator 'ones' (=32, matching the x32 wv prescale) never
            # change: write them all here while the vector engine is idle
            # instead of inside the qt0 attention weave
            for t2 in range(NTOK // 2):
                vt_all = v2_sb[t2].rearrange("p s (g c) -> p (s g) c", c=128)
                nc.vector.memset(vt_all[:, :, 64:128], 32.0)
            # ctx^T (normalized, bf16): chunk p = batch p, partitions 0:64 =
            # local head 0, 64:128 = local head 1, cols = batch p's tokens
            ctxT_sb = [
                persist.tile([128, S], dt.float8e4, name=f"ctxT{p}")
                for p in range(PAIRS)
            ]

            # ---------------- QK projection (c-outer, all 8 PSUM banks) -------
            # qk_ps region idx = dm*4 + nh*2 + j accumulates over c; iteration c
            # only needs hsT chunk c + wqk chunk c, so compute starts while the
            # rest of the inputs are still in flight. m-chunk m: batch m//2,
            # K/Q = m%2 (wqk cols (m%2)*128).
            with tc.tile_pool(name="psqk", bufs=1, space="PSUM") as psqk:
                for mp in range(2):  # m-pass: m in {2mp, 2mp+1} = batch mp
                    qk_ps = psqk.tile([128, 8, 512], dt.float32, name="qk_ps")
                    for cp in range(NCD // 2):
                        for dm in range(2):
                            m = 2 * mp + dm
                            for nh in range(2):
                                for j in range(2):
                                    nc.tensor.matmul(
                                        qk_ps[:, dm * 4 + nh * 2 + j, :],
                                        lhsT=wqk_all[
                                            :,
                                            2 * cp : 2 * cp + 2,
                                            dm * 128 : (dm + 1) * 128,
                                        ],
                                        rhs=hsT_all[
                                            :,
                                            2 * cp : 2 * cp + 2,
                                            mp * 2048
                                            + nh * 1024
                                            + j * 512 : mp * 2048
                                            + nh * 1024
                                            + (j + 1) * 512,
                                        ],
                                        start=(cp == 0),
                                        stop=(cp == NCD // 2 - 1),
                                        perf_mode=mybir.MatmulPerfMode.DoubleRow,
                                    )
                    for dm in range(2):
                        m = 2 * mp + dm
                        for nh in range(2):
                            nc.scalar.activation(
                                out=qkT_sb[m][:, nh * 1024 : (nh + 1) * 1024],
                                in_=qk_ps[
                                    :, dm * 4 + nh * 2 : dm * 4 + nh * 2 + 2, :
                                ],
                                func=Act.Identity,
                                bias=bqk_sb[dm],
                            )

            # psqk released; attention pools take over PSUM
            with (
                tc.tile_pool(name="psmm", bufs=3, space="PSUM") as psmm,
                tc.tile_pool(name="psctx", bufs=1, space="PSUM") as psctx,
            ):
                # V[tc] group g=2b+l: cols l*... ps[:, b*128+l*64 : +64] =
                # hs[b, tok_chunk] @ wv[:, l*64:...]; v tile cols g*128+64 :
                # (g+1)*128 are constant 32.0 (denominator trick)
                def emit_v_chunk(t):
                    ps = psmm.tile([128, 1024], dt.float32, name="ps_mm")
                    for b in range(2):
                        for cp in range(NCD // 2):
                            nc.tensor.matmul(
                                ps[:, b * 128 : (b + 1) * 128],
                                lhsT=hsT_all[
                                    :,
                                    2 * cp : 2 * cp + 2,
                                    b * 2048 + t * 128 : b * 2048 + (t + 1) * 128,
                                ],
                                rhs=wv_all[:, 2 * cp : 2 * cp + 2, :],
                                start=(cp == 0),
                                stop=(cp == NCD // 2 - 1),
                                perf_mode=mybir.MatmulPerfMode.DoubleRow,
                            )
                    vt = v2_sb[t // 2][:, t % 2, :].rearrange(
                        "p (g c) -> p g c", c=128
                    )
                    nc.vector.tensor_copy(
                        out=vt[:, :, 0:64],
                        in_=ps[:, 0:256].rearrange("p (g c) -> p g c", c=64),
                    )

                for t in range(4):
                    emit_v_chunk(t)

                # ------------- phase 2: attention + A2A + dense + LN ----------
                # q-tile-major; after each q-tile's ctx is normalized, the
                # chunk's ctxT slices are DMAed out and an 8-core AllToAll
                # fires. dense+LN for qt-1 are woven into qt's second (p==1)
                # attention pass, by which point A2A(qt-1) has long landed.
                def emit_a2a(qt, b):
                    # batch b's ctxT slice is final right after pass p=b's
                    # normalize: stage it immediately; trigger per-half for
                    # the last q-tile, once per q-tile otherwise
                    if qt < NQT - 1:
                        cin = cc_in[qt][0][:, :].rearrange(
                            "(r c) (bb t) -> c r bb t", c=128, t=64
                        )[:, :, b, :]
                    else:
                        cin = cc_in[qt][b][:, :].rearrange(
                            "(r c) t -> c r t", c=128
                        )
                    nc.sync.dma_start(
                        out=cin,
                        in_=ctxT_sb[b][
                            :, qt * 512 : (qt + 1) * 512
                        ].rearrange("c (r t) -> c r t", t=64),
                    )
                    if qt == NQT - 1 or b == 1:
                        idx = b if qt == NQT - 1 else 0
                        nc.gpsimd.collective_compute(
                            "AllToAll",
                            Alu.bypass,
                            replica_groups=REPLICA_GROUPS,
                            ins=[cc_in[qt][idx][:, :].opt()],
                            outs=[cc_out[qt][idx][:, :].opt()],
                        )

                # dense + residual + LN for one q-tile's 128-token shard,
                # staged so each piece slots into engine slack of the covering
                # attention pass (fetch / matmul blob+evac / stats / finish)
                dense_state = {}

                def emit_ctx_fetch(qt):
                    ctx_sb = work.tile([128, NCD, 128], dt.float8e4, name="ctx_sb")
                    if qt < NQT - 1:
                        nc.sync.dma_start(
                            out=ctx_sb,
                            in_=cc_out[qt][0][:, :].rearrange(
                                "(c p) t -> p c t", p=128
                            ),
                        )
                    else:
                        for b in range(2):
                            nc.sync.dma_start(
                                out=ctx_sb[:, :, b * 64 : (b + 1) * 64],
                                in_=cc_out[qt][b][:, :].rearrange(
                                    "(c p) t -> p c t", p=128
                                ),
                            )
                    dense_state["ctx_sb"] = ctx_sb

                def emit_dense(qt):
                    # 16 matmuls + immediate add-evacuation (x = dense + res).
                    # ps_d's full lifetime is inside this call, so sharing the
                    # ps_mm rotation with the scores pipeline is safe.
                    ctx_sb = dense_state["ctx_sb"]
                    ps_d = psmm.tile([128, 1024], dt.float32, name="ps_mm")
                    for cp in range(NCD // 2):
                        for j in range(2):
                            nc.tensor.matmul(
                                ps_d[:, j * 512 : (j + 1) * 512],
                                lhsT=ctx_sb[:, 2 * cp : 2 * cp + 2, :],
                                rhs=wd_all[:, 2 * cp : 2 * cp + 2, j * 512 : (j + 1) * 512],
                                start=(cp == 0),
                                stop=(cp == NCD // 2 - 1),
                                perf_mode=mybir.MatmulPerfMode.DoubleRow,
                            )
                    x = lnp.tile([128, HIDDEN], dt.float32, name="x")
                    nc.vector.tensor_tensor(
                        out=x, in0=ps_d, in1=res_all[:, qt, :], op=Alu.add
                    )
                    dense_state["x"] = x

                def emit_ln_stats(qt):
                    x = dense_state["x"]
                    stats = lnp.tile([128, 2, 6], dt.float32, name="stats")
                    xv = x.rearrange("p (s f) -> p s f", f=512)
                    for i in range(2):
                        nc.vector.bn_stats(out=stats[:, i, :], in_=xv[:, i, :])
                    mv = lnp.tile([128, 2], dt.float32, name="mv")
                    nc.vector.bn_aggr(out=mv, in_=stats)
                    dense_state["mv"] = mv

                def emit_ln_fin(qt):
                    x = dense_state["x"]
                    mv = dense_state["mv"]
                    # rstd = exp(-0.5 * ln(var + eps)) -- stays in the exp/ln tables
                    lnv = lnp.tile([128, 1], dt.float32, name="lnv")
                    nc.scalar.activation(
                        out=lnv, in_=mv[:, 1:2], func=Act.Ln, bias=eps_sb
                    )
                    rstd = lnp.tile([128, 1], dt.float32, name="rstd")
                    nc.scalar.activation(
                        out=rstd, in_=lnv, func=Act.Exp, scale=-0.5
                    )
                    y = lnp.tile([128, HIDDEN], dt.float32, name="y")
                    nc.vector.tensor_scalar(
                        out=y,
                        in0=x,
                        scalar1=mv[:, 0:1],
                        scalar2=rstd,
                        op0=Alu.subtract,
                        op1=Alu.mult,
                    )
                    nc.sync.dma_start(
                        out=out[qt * 128 : (qt + 1) * 128, :], in_=y
                    )

                WEAVE = {
                    (2, 0, 10): (emit_ctx_fetch, 0),
                    (2, 0, 14): (emit_dense, 0),
                    (2, 1, 4): (emit_ln_stats, 0),
                    (2, 1, 8): (emit_ln_fin, 0),
                    (2, 1, 14): (emit_ctx_fetch, 1),
                    (3, 0, 2): (emit_dense, 1),
                    (3, 0, 6): (emit_ln_stats, 1),
                    (3, 0, 10): (emit_ln_fin, 1),
                    (3, 1, 6): (emit_ctx_fetch, 2),
                    (3, 1, 10): (emit_dense, 2),
                    (3, 1, 14): (emit_ln_stats, 2),
                }

                for qt in range(NQT):
                    for p in range(PAIRS):
                        km = 2 * p  # K m-chunk (batch p)
                        qm = 2 * p + 1  # Q m-chunk (batch p)
                        ctx_ps = [
                            psctx.tile([128, 512], dt.float32, name=f"ps_ctx{l}")
                            for l in range(2)
                        ]

                        def emit_scores(kc, km=km, qm=qm, qt=qt):
                            ps_s = psmm.tile([128, 1024], dt.float32, name="ps_mm")
                            # scores^T for both local heads (concurrent row
                            # groups: head0 rows 0:64, head1 rows 64:128)
                            for l in range(2):
                                nc.tensor.matmul(
                                    ps_s[:, l * 512 : (l + 1) * 512],
                                    lhsT=qkT_sb[km][
                                        l * 64 : (l + 1) * 64, kc * 128 : (kc + 1) * 128
                                    ],
                                    rhs=qkT_sb[qm][
                                        l * 64 : (l + 1) * 64, qt * 512 : (qt + 1) * 512
                                    ],
                                    start=True,
                                    stop=True,
                                    tile_position=(l * 64, 0),
                                )
                            return ps_s

                        # software pipeline: scores run one k-chunk ahead so the
                        # PE never sits in-order behind ctx(k)'s wait on exp(k).
                        # probs are written as fp8 in kc pairs; each pair is one
                        # DoubleRow ctx matmul (2 fp8 weights/cell, K=256).
                        ps_s = emit_scores(0)
                        pT2 = None
                        for kc in range(NTOK):
                            kc2, sl = kc // 2, kc % 2
                            if sl == 0:
                                pT2 = pT_pool.tile(
                                    [128, 2, 1024], dt.float8e4, name="pT2"
                                )
                            ps_s_next = emit_scores(kc + 1) if kc + 1 < NTOK else None
                            if kc in DVE_KC:
                                # vector-engine poly exp (frees the ACT engine)
                                nc.vector._custom_dve(
                                    EXP_OP,
                                    out=pT2[:, sl, :],
                                    in0=ps_s,
                                    s0=EXP_S0,
                                    s1=EXP_S1,
                                    imm2=EXP_IMM2,
                                )
                            else:
                                nc.scalar.activation(
                                    out=pT2[:, sl, :],
                                    in_=ps_s,
                                    func=Act.Exp,
                                    scale=0.125 / 1024,
                                )
                            ps_s = ps_s_next
                            # ctx^T (+ denominator rows 64:128): one DoubleRow
                            # matmul per kc pair per head, accumulated over kc2
                            if sl == 1:
                                for l in range(2):
                                    g = 2 * p + l
                                    nc.tensor.matmul(
                                        ctx_ps[l],
                                        lhsT=v2_sb[kc2][
                                            :, :, g * 128 : (g + 1) * 128
                                        ],
                                        rhs=pT2[:, :, l * 512 : (l + 1) * 512],
                                        start=(kc2 == 0),
                                        stop=(kc2 == NTOK // 2 - 1),
                                        perf_mode=mybir.MatmulPerfMode.DoubleRow,
                                    )
                            # first q-tile: produce the remaining V chunks just
                            # ahead of their use (ctx(kc) needs v_sb[kc]); later
                            # q-tiles: weave previous q-tiles' dense+LN stages
                            # (which consume those q-tiles' A2As) per WEAVE.
                            # The pipeline runs ~1.5 q-tiles behind attention:
                            # the early collectives are 2-3x slower than steady
                            # state, and a fetch dispatched before its A2A
                            # completes would block the in-order sync queue
                            # (delaying the next q-tile's staging DMAs).
                            if p == 0 and qt == 0 and kc + 4 < NTOK:
                                emit_v_chunk(kc + 4)
                            act = WEAVE.get((qt, p, kc))
                            if act is not None:
                                fn, dqt = act
                                fn(dqt)
                        # normalize: ctx[0:64] / den[64:128] -> ctxT (fp8);
                        # both heads' denominators share one reciprocal pass
                        # (reciprocal_approx_fast must NOT read PSUM directly:
                        # that produced NaNs; the SBUF den2 copy is load-bearing)
                        den2 = work.tile([128, 512], dt.float32, name="den2")
                        for l in range(2):
                            nc.vector.tensor_copy(
                                out=den2[l * 64 : (l + 1) * 64, :],
                                in_=ctx_ps[l][64:128, :],
                            )
                        rec = work.tile([128, 512], dt.float32, name="rec")
                        nc.vector.reciprocal_approx_fast(out=rec, in_=den2)
                        for l in range(2):
                            nc.vector.tensor_tensor(
                                out=ctxT_sb[p][
                                    l * 64 : (l + 1) * 64, qt * 512 : (qt + 1) * 512
                                ],
                                in0=ctx_ps[l][0:64, :],
                                in1=rec[l * 64 : (l + 1) * 64, :],
                                op=Alu.mult,
                            )
                        emit_a2a(qt, p)
                # last q-tile's dense+LN have no following attention to hide
                # in; qt2's LN tail fills the final exchange's flight time
                emit_ln_fin(NQT - 2)
                emit_ctx_fetch(NQT - 1)
                emit_dense(NQT - 1)
                emit_ln_stats(NQT - 1)
                emit_ln_fin(NQT - 1)

    nc.compile()
    return nc


_PROGRAM = None


def _get_program():
    global _PROGRAM
    if _PROGRAM is None:
        _PROGRAM = _build_program()
    return _PROGRAM


def _prep_core_inputs(hidden_states, w_qkv, b_qkv, w_dense, b_dense):
    """Build the 8 per-core input maps (numpy, host-side sharding)."""
    hs = np.asarray(hidden_states, dtype=np.float32)
    w_qkv = np.asarray(w_qkv, dtype=np.float32)
    b_qkv = np.asarray(b_qkv, dtype=np.float32)
    w_dense = np.asarray(w_dense, dtype=np.float32)
    b_dense = np.asarray(b_dense, dtype=np.float32)

    # v-channel bias folded into a host-side output bias:
    # b_out = b_dense + b_v_full @ w_dense   (b_v in ctx channel order)
    bv_full = np.empty((HIDDEN,), dtype=np.float64)
    for g in range(HEADS):
        bv_full[g * HD : (g + 1) * HD] = b_qkv[g * 192 + 128 : g * 192 + 192]
    # w_dense rows are already in (head, d) = g*64+d order, matching bv_full
    b_out = (
        b_dense.astype(np.float64)
        + bv_full @ w_dense.astype(np.float64)
    ).astype(np.float32)

    # shared across cores: both batches' hs^T in fp8, full dense weight
    hsT_bf = np.concatenate(
        [np.ascontiguousarray(hs[0].T), np.ascontiguousarray(hs[1].T)], axis=1
    ).astype(FP8)  # [1024, 4096]
    # x256 prescale keeps wd in fp8e4 normal range; the dense partials come
    # out x256 and the residual is prescaled to match (LN is scale-invariant)
    wd_bf = (w_dense * 256).astype(FP8)  # [1024, 1024], rows channel-ordered

    in_maps = []
    for r in range(N_CORES):
        gheads = [2 * r, 2 * r + 1]

        # wqk column order: K h0 | K h1 | Q h0 | Q h1 (64 each)
        wqk_cols = np.empty((HIDDEN, 256), dtype=np.float32)
        bqk_vec = np.empty((256,), dtype=np.float32)
        for l, g in enumerate(gheads):
            kcol = slice(g * 192 + 64, g * 192 + 128)
            qcol = slice(g * 192, g * 192 + 64)
            wqk_cols[:, l * 64 : (l + 1) * 64] = w_qkv[:, kcol]
            wqk_cols[:, 128 + l * 64 : 128 + (l + 1) * 64] = w_qkv[:, qcol]
            bqk_vec[l * 64 : (l + 1) * 64] = b_qkv[kcol]
            bqk_vec[128 + l * 64 : 128 + (l + 1) * 64] = b_qkv[qcol]

        wv_cols = np.empty((HIDDEN, 128), dtype=np.float32)
        for l, g in enumerate(gheads):
            wv_cols[:, l * 64 : (l + 1) * 64] = w_qkv[
                :, g * 192 + 128 : g * 192 + 192
            ]

        # residual shard (+ folded output bias): row qt*128 + b*64 + t
        # covers full[b, qt*512 + r*64 + t]
        res = np.empty((OUT_ROWS, HIDDEN), dtype=np.float32)
        for qt in range(NQT):
            t0 = qt * 512 + r * 64
            for b in range(B):
                res[qt * 128 + b * 64 : qt * 128 + (b + 1) * 64, :] = 256 * (
                    hs[b, t0 : t0 + 64, :] + b_out
                )

        in_maps.append(
            {
                "hsT": hsT_bf,
                "wqk": (wqk_cols * 32).astype(FP8),
                "wv": (wv_cols * 32).astype(FP8),
                "wd": wd_bf,
                "bqk": (bqk_vec * 32).reshape(256, 1),
                "hs_res": res.astype(BF16),
            }
        )
    return in_maps


def kernel(hidden_states, w_qkv, b_qkv, w_dense, b_dense, ln_gamma, ln_beta,
           _return_perf=False, **run_kwargs):
    ln_gamma = np.asarray(ln_gamma, dtype=np.float32)
    ln_beta = np.asarray(ln_beta, dtype=np.float32)
    gamma_one = np.allclose(ln_gamma, 1.0)
    beta_zero = np.allclose(ln_beta, 0.0)

    nc = _get_program()
    in_maps = _prep_core_inputs(hidden_states, w_qkv, b_qkv, w_dense, b_dense)
    res = run_bass_kernel_spmd(
        nc, in_maps, core_ids=list(range(N_CORES)), **run_kwargs
    )

    full = np.empty((B, S, HIDDEN), dtype=np.float32)
    for r in range(N_CORES):
        o = res.results[r]["out"]
        for qt in range(NQT):
            t0 = qt * 512 + r * 64
            for b in range(B):
                full[b, t0 : t0 + 64, :] = o[
                    qt * 128 + b * 64 : qt * 128 + (b + 1) * 64, :
                ]

    if not (gamma_one and beta_zero):
        # spec fills gamma=ones, beta=zeros; fall back on host if they differ
        full = full * ln_gamma[None, None, :] + ln_beta[None, None, :]

    if _return_perf:
        return full, res
    return full


# revision 29
# speedup vs baseline: 1.2084x; 1.0013x over previous
"""BERT self-attention block (QKV -> attention -> dense -> residual+LN) on 8 trn2 NeuronCores.

Sharding: tensor-parallel over heads across all 8 cores (2 heads/core), with BOTH
batch elements on every core (batch plays the old "head pair" role in the attention
weave). After attention, a per-q-tile 8-core AllToAll exchanges ctx^T (bf16,
256KB/chunk) so each core owns the full 1024 ctx channels for its 128-token shard
of the chunk; the core then computes the full dense projection + residual + LN
locally and the host reassembles the [2, 2048, 1024] output.

This replaces the old scheme (DP batch x TP=4 heads, dense partials summed with a
chunked ReduceScatter) whose serialized CC chain (4MB/core at ~25GB/s = ~156us)
dominated the tail: the A2A moves 4x fewer bytes and fires right after each
q-tile's attention instead of waiting for dense.

Perf structure (inherited from the tuned baseline):
- softmax exp is split between the scalar engine (ACT spline exp) and a
  custom vector-engine op (quadratic poly + 4 squarings ~= exp(x/8)),
  alternating engines within each kc pair so both probs of a pair finish
  together;
- probs are written as fp8e4 in kc pairs and each pair is one DoubleRow
  ctx matmul (2 fp8 weights/cell, K=256) -- halves the ctx PE slots;
- the scores PSUM pool is triple-buffered (with dense/V tiles rotating
  through the same pool) so the PE streams without >3.4us idle windows
  that would re-throttle the HAM clock gate to 4/8;
- QKV projections run in fp8 DoubleRow (weights prescaled x32 on the host,
  the resulting x1024 score scale folded into the exp constants, denominator
  'ones' set to 32.0 so the softmax ratio is unscaled), c-outer so compute
  starts while input DMAs are in flight; qkT bias-evacuation runs on the
  then-idle scalar engine via Act.Identity's per-partition bias.
"""

import sys

for _p in ("/opt/trn_rl_repo",):
    if _p not in sys.path:
        sys.path.insert(0, _p)

import numpy as np
import ml_dtypes

import concourse.bass as bass
import concourse.mybir as mybir
import concourse.tile as tile
from concourse import bacc
from concourse.bass_utils import run_bass_kernel_spmd

BF16 = ml_dtypes.bfloat16
FP8 = ml_dtypes.float8_e4m3

HIDDEN = 1024
HEADS = 16
HD = 64  # head dim
B = 2
S = 2048
LN_EPS = 1e-5

N_CORES = 8
LHEADS = 2  # heads per core
PAIRS = 2  # attention passes per q-tile: pair p = batch p (2 local heads each)
NCD = HIDDEN // 128  # 8 contraction chunks
NTOK = S // 128  # 16 token chunks (per batch)
NQT = 4  # attention q-tiles (512 q each)
QT = S // NQT  # 512
REPLICA_GROUPS = [[0, 1, 2, 3, 4, 5, 6, 7]]
# per-core output: for each q-tile, 64 tokens of each batch
# (rows qt*128 + b*64 + t  <->  full[b, qt*512 + rank*64 + t])
OUT_ROWS = NQT * 128  # 512

# which kc chunks the vector engine handles for exp (rest go to ACT)
DVE_KC = frozenset((1, 3, 5, 7, 11, 13))

dt = mybir.dt
Alu = mybir.AluOpType
Act = mybir.ActivationFunctionType

# ---------------- custom DVE op: poly exp ----------------
# out = (imm2 + x*(s0 + x*s1))^16  ~=  exp(x/8) for x in +-28 (raw q.k scores)
# (quadratic fit of exp(t) on t = x/128 in +-0.225, then 4 squarings)
_CQ = (1.00004518, 1.00351622, 0.49634025)
EXP_S0 = float(_CQ[1] / 128 / 1024)
EXP_S1 = float(_CQ[2] / (128 * 128) / (1024 * 1024))
EXP_IMM2 = float(_CQ[0])


def _register_exp_op():
    from concourse import dve_ops as DO
    from concourse.dve_spec import Spec, Src0, C0, C1, C2, lower
    from concourse.dve_spec import _has_src1 as has_src1
    from concourse.dve_uop import DveOpSpec

    name = "EXP_Q4_ANT"
    for o in DO.OPS:
        if o.name == name:
            return o
    a1 = Src0 * C1 + C0
    a2 = Src0 * a1 + C2
    p2 = a2 * a2
    p4 = p2 * p2
    p8 = p4 * p4
    body = p8 * p8

    def _ref(in0, in1, s0, s1, imm2):
        p = imm2 + in0 * (s0 + in0 * s1)
        for _ in range(4):
            p = p * p
        return p

    spec = Spec(body=body, reference=_ref)
    row = DO._CUSTOM_DVE_ROW_BASE + len(DO.OPS)
    DO._SUB_OPCODE_FOR_NAME[name] = row
    shas = {}
    for ver in ("v3", "v4"):
        uops = lower(spec, ver=ver)
        shas[ver] = DveOpSpec(
            name=name, opcode=row, uops=uops, rd1_en=has_src1(spec)
        ).sha(ver)
    op = DO.DveOp(name, spec, subdim=False, uops_sha=shas)
    DO.OPS.append(op)
    DO.CUSTOM_DVE_SPECS[name] = spec
    return op


EXP_OP = _register_exp_op()


def _build_program():
    nc = bacc.Bacc(
        "TRN2", target_bir_lowering=False, debug=False, num_devices=N_CORES
    )

    # Route Exp and Ln to the one table set that holds both, so the kernel
    # never reloads ACT tables (set ids are positional; only values change).
    from concourse import hw_specs

    for name, funcs in hw_specs.get_activation_tables(nc.m.arch).items():
        if name != "natural_log_exp_and_others":
            funcs.discard(Act.Exp)
            funcs.discard(Act.Ln)

    # ---------------- DRAM I/O ----------------
    # hsT: both batches, [1024, 4096] = [hid, b*2048 + t]
    hsT = nc.dram_tensor("hsT", [HIDDEN, B * S], dt.float8e4, kind="ExternalInput")
    # wqk: [1024, 256] = [K h0 | K h1 | Q h0 | Q h1] (x32 prescale)
    wqk = nc.dram_tensor("wqk", [HIDDEN, 256], dt.float8e4, kind="ExternalInput")
    # wv: [1024, 128] = [V h0 | V h1] (x32 prescale)
    wv = nc.dram_tensor("wv", [HIDDEN, 128], dt.float8e4, kind="ExternalInput")
    # wd: full dense weight [1024, 1024]
    wd = nc.dram_tensor("wd", [HIDDEN, HIDDEN], dt.float8e4, kind="ExternalInput")
    bqk = nc.dram_tensor("bqk", [256, 1], dt.float32, kind="ExternalInput")
    # residual (+ folded dense bias) for this core's token shard
    hs_res = nc.dram_tensor(
        "hs_res", [OUT_ROWS, HIDDEN], dt.bfloat16, kind="ExternalInput"
    )
    out = nc.dram_tensor("out", [OUT_ROWS, HIDDEN], dt.float32, kind="ExternalOutput")

    # internal DRAM for the collective (cannot use I/O tensors)
    # cc layout per qt: [8 peer blocks * 128 chan, 128] where block r =
    # my 128 channels for tokens qt*512 + r*64 (+64 of each batch:
    # cols 0:64 = batch0, 64:128 = batch1). NOTE: a [64, 2048] variant
    # (4KB rows) measured SLOWER -- the CC mesh parallelizes across rows,
    # so keep many rows.
    # qt0-2 exchange once per q-tile (fewer sync-queue wait points in the
    # steady state); the LAST q-tile splits per batch so the tail only
    # waits on a 64KB op and batch0's half hides under the p==1 pass
    cc_in = [
        [nc.dram_tensor(f"cc_in{q}", [N_CORES * 128, 128], dt.float8e4)]
        if q < NQT - 1
        else [
            nc.dram_tensor(f"cc_in{q}_{b}", [N_CORES * 128, 64], dt.float8e4)
            for b in range(2)
        ]
        for q in range(NQT)
    ]
    cc_out = [
        [nc.dram_tensor(f"cc_out{q}", [N_CORES * 128, 128], dt.float8e4)]
        if q < NQT - 1
        else [
            nc.dram_tensor(f"cc_out{q}_{b}", [N_CORES * 128, 64], dt.float8e4)
            for b in range(2)
        ]
        for q in range(NQT)
    ]
    # tiny dummy exchange fired at kernel start: absorbs the ~11.5us
    # first-collective trigger delay + CC DMA-ring spin-up so A2A(qt0)
    # runs at warm-stream speed
    cc_warm_in = nc.dram_tensor("cc_warm_in", [N_CORES, 128], dt.bfloat16)
    cc_warm_out = nc.dram_tensor("cc_warm_out", [N_CORES, 128], dt.bfloat16)

    with tile.TileContext(nc) as tc:
        with (
            tc.tile_pool(name="persist", bufs=1) as persist,
            tc.tile_pool(name="pT_pool", bufs=6) as pT_pool,
            tc.tile_pool(name="work", bufs=3) as work,
            tc.tile_pool(name="ln", bufs=2) as lnp,
        ):
            # ---------------- persistent SBUF loads ----------------
            zero_sb = persist.tile([128, 1], dt.float32, name="zero_sb")
            nc.vector.memset(zero_sb, 0.0)
            nc.const_aps.aps[(dt.float32, 0.0)] = zero_sb
            eps_sb = persist.tile([128, 1], dt.float32, name="eps_sb")
            nc.vector.memset(eps_sb, LN_EPS)
            # warm the CC stream before any data dep can delay the trigger
            nc.gpsimd.collective_compute(
                "AllToAll",
                Alu.bypass,
                replica_groups=REPLICA_GROUPS,
                ins=[cc_warm_in[:, :].opt()],
                outs=[cc_warm_out[:, :].opt()],
            )
            # input DMAs: interleaved so the c-outer QK matmuls can start
            # after the first hsT/wqk chunk pair lands (the sync queue
            # serializes at ~0.6us per dma_start dispatch, so keep them few).
            # wd/res (3MB) aren't consumed until ~120us in: dispatch them
            # last so they don't steal HBM bandwidth from the hsT stream.
            hsT_all = persist.tile([128, NCD, B * S], dt.float8e4, name="hsT_all")
            hsT_r = hsT[:, :].rearrange("(c p) t -> p c t", p=128)
            wqk_all = persist.tile([128, NCD, 256], dt.float8e4, name="wqk_all")
            wqk_r = wqk[:, :].rearrange("(c p) n -> p c n", p=128)
            nc.sync.dma_start(out=wqk_all[:, 0:8, :], in_=wqk_r[:, 0:8, :])
            for c in range(NCD):
                nc.sync.dma_start(
                    out=hsT_all[:, c : c + 1, :], in_=hsT_r[:, c : c + 1, :]
                )
            bqk_all = persist.tile([128, 2], dt.float32, name="bqk_all")
            nc.sync.dma_start(
                out=bqk_all, in_=bqk[:, :].rearrange("(m p) o -> p (m o)", p=128)
            )
            wv_all = persist.tile([128, NCD, 128], dt.float8e4, name="wv_all")
            nc.sync.dma_start(
                out=wv_all, in_=wv[:, :].rearrange("(c p) n -> p c n", p=128)
            )
            wd_all = persist.tile([128, NCD, HIDDEN], dt.float8e4, name="wd_all")
            nc.sync.dma_start(
                out=wd_all, in_=wd[:, :].rearrange("(c p) n -> p c n", p=128)
            )
            res_all = persist.tile([128, NQT, HIDDEN], dt.bfloat16, name="res_all")
            nc.sync.dma_start(
                out=res_all,
                in_=hs_res[:, :].rearrange("(g p) n -> p g n", p=128),
            )
            bqk_sb = [bqk_all[:, m : m + 1] for m in range(2)]

            # qkT m-chunk layout: 0=K batch0, 1=Q batch0, 2=K batch1, 3=Q batch1
            # (partitions 0:64 = local head 0, 64:128 = local head 1)
            qkT_sb = [
                persist.tile([128, S], dt.bfloat16, name=f"qkT{m}") for m in range(4)
            ]
            # V tiles (fp8, DoubleRow pairs): tile t2 slot s covers token
            # chunk 2*t2+s as 4 groups (g = 2*batch + head) of [V_h(64) | ones(64)]
            v2_sb = [
                persist.tile([128, 2, 512], dt.float8e4, name=f"v{t2}")
                for t2 in range(NTOK // 2)
            ]
            # the denominator 'ones' (=32, matching the x32 wv prescale) never
            # change: write them all here while the vector engine is idle
            # instead of inside the qt0 attention weave
            for t2 in range(NTOK // 2):
                vt_all = v2_sb[t2].rearrange("p s (g c) -> p (s g) c", c=128)
                nc.vector.memset(vt_all[:, :, 64:128], 32.0)
            # ctx^T (normalized, bf16): chunk p = batch p, partitions 0:64 =
            # local head 0, 64:128 = local head 1, cols = batch p's tokens
            ctxT_sb = [
                persist.tile([128, S], dt.float8e4, name=f"ctxT{p}")
                for p in range(PAIRS)
            ]

            # ---------------- QK projection (c-outer, all 8 PSUM banks) -------
            # qk_ps region idx = dm*4 + nh*2 + j accumulates over c; iteration c
            # only needs hsT chunk c + wqk chunk c, so compute starts while the
            # rest of the inputs are still in flight. m-chunk m: batch m//2,
            # K/Q = m%2 (wqk cols (m%2)*128).
            with tc.tile_pool(name="psqk", bufs=1, space="PSUM") as psqk:
                for mp in range(2):  # m-pass: m in {2mp, 2mp+1} = batch mp
                    qk_ps = psqk.tile([128, 8, 512], dt.float32, name="qk_ps")
                    for cp in range(NCD // 2):
                        for dm in range(2):
                            m = 2 * mp + dm
                            for nh in range(2):
                                for j in range(2):
                                    nc.tensor.matmul(
                                        qk_ps[:, dm * 4 + nh * 2 + j, :],
                                        lhsT=wqk_all[
                                            :,
                                            2 * cp : 2 * cp + 2,
                                            dm * 128 : (dm + 1) * 128,
                                        ],
                                        rhs=hsT_all[
                                            :,
                                            2 * cp : 2 * cp + 2,
                                            mp * 2048
                                            + nh * 1024
                                            + j * 512 : mp * 2048
                                            + nh * 1024
                                            + (j + 1) * 512,
                                        ],
                                        start=(cp == 0),
                                        stop=(cp == NCD // 2 - 1),
                                        perf_mode=mybir.MatmulPerfMode.DoubleRow,
                                    )
                    for dm in range(2):
                        m = 2 * mp + dm
                        for nh in range(2):
                            nc.scalar.activation(
                                out=qkT_sb[m][:, nh * 1024 : (nh + 1) * 1024],
                                in_=qk_ps[
                                    :, dm * 4 + nh * 2 : dm * 4 + nh * 2 + 2, :
                                ],
                                func=Act.Identity,
                                bias=bqk_sb[dm],
                            )

            # psqk released; attention pools take over PSUM
            with (
                tc.tile_pool(name="psmm", bufs=3, space="PSUM") as psmm,
                tc.tile_pool(name="psctx", bufs=1, space="PSUM") as psctx,
            ):
                # V[tc] group g=2b+l: cols l*... ps[:, b*128+l*64 : +64] =
                # hs[b, tok_chunk] @ wv[:, l*64:...]; v tile cols g*128+64 :
                # (g+1)*128 are constant 32.0 (denominator trick)
                def emit_v_chunk(t):
                    ps = psmm.tile([128, 1024], dt.float32, name="ps_mm")
                    for b in range(2):
                        for cp in range(NCD // 2):
                            nc.tensor.matmul(
                                ps[:, b * 128 : (b + 1) * 128],
                                lhsT=hsT_all[
                                    :,
                                    2 * cp : 2 * cp + 2,
                                    b * 2048 + t * 128 : b * 2048 + (t + 1) * 128,
                                ],
                                rhs=wv_all[:, 2 * cp : 2 * cp + 2, :],
                                start=(cp == 0),
                                stop=(cp == NCD // 2 - 1),
                                perf_mode=mybir.MatmulPerfMode.DoubleRow,
                            )
                    vt = v2_sb[t // 2][:, t % 2, :].rearrange(
                        "p (g c) -> p g c", c=128
                    )
                    nc.vector.tensor_copy(
                        out=vt[:, :, 0:64],
                        in_=ps[:, 0:256].rearrange("p (g c) -> p g c", c=64),
                    )

                for t in range(4):
                    emit_v_chunk(t)

                # ------------- phase 2: attention + A2A + dense + LN ----------
                # q-tile-major; after each q-tile's ctx is normalized, the
                # chunk's ctxT slices are DMAed out and an 8-core AllToAll
                # fires. dense+LN for qt-1 are woven into qt's second (p==1)
                # attention pass, by which point A2A(qt-1) has long landed.
                def emit_a2a(qt, b):
                    # batch b's ctxT slice is final right after pass p=b's
                    # normalize: stage it immediately; trigger per-half for
                    # the last q-tile, once per q-tile otherwise
                    if qt < NQT - 1:
                        cin = cc_in[qt][0][:, :].rearrange(
                            "(r c) (bb t) -> c r bb t", c=128, t=64
                        )[:, :, b, :]
                    else:
                        cin = cc_in[qt][b][:, :].rearrange(
                            "(r c) t -> c r t", c=128
                        )
                    nc.sync.dma_start(
                        out=cin,
                        in_=ctxT_sb[b][
                            :, qt * 512 : (qt + 1) * 512
                        ].rearrange("c (r t) -> c r t", t=64),
                    )
                    if qt == NQT - 1 or b == 1:
                        idx = b if qt == NQT - 1 else 0
                        nc.gpsimd.collective_compute(
                            "AllToAll",
                            Alu.bypass,
                            replica_groups=REPLICA_GROUPS,
                            ins=[cc_in[qt][idx][:, :].opt()],
                            outs=[cc_out[qt][idx][:, :].opt()],
                        )

                # dense + residual + LN for one q-tile's 128-token shard,
                # staged so each piece slots into engine slack of the covering
                # attention pass (fetch / matmul blob+evac / stats / finish)
                dense_state = {}

                def emit_ctx_fetch(qt):
                    ctx_sb = work.tile([128, NCD, 128], dt.float8e4, name="ctx_sb")
                    if qt < NQT - 1:
                        nc.sync.dma_start(
                            out=ctx_sb,
                            in_=cc_out[qt][0][:, :].rearrange(
                                "(c p) t -> p c t", p=128
                            ),
                        )
                    else:
                        for b in range(2):
                            nc.sync.dma_start(
                                out=ctx_sb[:, :, b * 64 : (b + 1) * 64],
                                in_=cc_out[qt][b][:, :].rearrange(
                                    "(c p) t -> p c t", p=128
                                ),
                            )
                    dense_state["ctx_sb"] = ctx_sb

                def emit_dense(qt):
                    # 16 matmuls + immediate add-evacuation (x = dense + res).
                    # ps_d's full lifetime is inside this call, so sharing the
                    # ps_mm rotation with the scores pipeline is safe.
                    ctx_sb = dense_state["ctx_sb"]
                    ps_d = psmm.tile([128, 1024], dt.float32, name="ps_mm")
                    for cp in range(NCD // 2):
                        for j in range(2):
                            nc.tensor.matmul(
                                ps_d[:, j * 512 : (j + 1) * 512],
                                lhsT=ctx_sb[:, 2 * cp : 2 * cp + 2, :],
                                rhs=wd_all[:, 2 * cp : 2 * cp + 2, j * 512 : (j + 1) * 512],
                                start=(cp == 0),
                                stop=(cp == NCD // 2 - 1),
                                perf_mode=mybir.MatmulPerfMode.DoubleRow,
                            )
                    x = lnp.tile([128, HIDDEN], dt.float32, name="x")
                    nc.vector.tensor_tensor(
                        out=x, in0=ps_d, in1=res_all[:, qt, :], op=Alu.add
                    )
                    dense_state["x"] = x

                def emit_ln_stats(qt):
                    x = dense_state["x"]
                    stats = lnp.tile([128, 2, 6], dt.float32, name="stats")
                    xv = x.rearrange("p (s f) -> p s f", f=512)
                    for i in range(2):
                        nc.vector.bn_stats(out=stats[:, i, :], in_=xv[:, i, :])
                    mv = lnp.tile([128, 2], dt.float32, name="mv")
                    nc.vector.bn_aggr(out=mv, in_=stats)
                    dense_state["mv"] = mv

                def emit_ln_fin(qt):
                    x = dense_state["x"]
                    mv = dense_state["mv"]
                    # rstd = exp(-0.5 * ln(var + eps)) -- stays in the exp/ln tables
                    lnv = lnp.tile([128, 1], dt.float32, name="lnv")
                    nc.scalar.activation(
                        out=lnv, in_=mv[:, 1:2], func=Act.Ln, bias=eps_sb
                    )
                    rstd = lnp.tile([128, 1], dt.float32, name="rstd")
                    nc.scalar.activation(
                        out=rstd, in_=lnv, func=Act.Exp, scale=-0.5
                    )
                    y = lnp.tile([128, HIDDEN], dt.float32, name="y")
                    nc.vector.tensor_scalar(
                        out=y,
                        in0=x,
                        scalar1=mv[:, 0:1],
                        scalar2=rstd,
                        op0=Alu.subtract,
                        op1=Alu.mult,
                    )
                    nc.sync.dma_start(
                        out=out[qt * 128 : (qt + 1) * 128, :], in_=y
                    )

                WEAVE = {
                    (2, 0, 10): (emit_ctx_fetch, 0),
                    (2, 0, 14): (emit_dense, 0),
                    (2, 1, 4): (emit_ln_stats, 0),
                    (2, 1, 8): (emit_ln_fin, 0),
                    (2, 1, 14): (emit_ctx_fetch, 1),
                    (3, 0, 2): (emit_dense, 1),
                    (3, 0, 6): (emit_ln_stats, 1),
                    (3, 0, 10): (emit_ln_fin, 1),
                    (3, 1, 2): (emit_ctx_fetch, 2),
                    (3, 1, 6): (emit_dense, 2),
                    (3, 1, 10): (emit_ln_stats, 2),
                }

                for qt in range(NQT):
                    for p in range(PAIRS):
                        km = 2 * p  # K m-chunk (batch p)
                        qm = 2 * p + 1  # Q m-chunk (batch p)
                        ctx_ps = [
                            psctx.tile([128, 512], dt.float32, name=f"ps_ctx{l}")
                            for l in range(2)
                        ]

                        def emit_scores(kc, km=km, qm=qm, qt=qt):
                            ps_s = psmm.tile([128, 1024], dt.float32, name="ps_mm")
                            # scores^T for both local heads (concurrent row
                            # groups: head0 rows 0:64, head1 rows 64:128)
                            for l in range(2):
                                nc.tensor.matmul(
                                    ps_s[:, l * 512 : (l + 1) * 512],
                                    lhsT=qkT_sb[km][
                                        l * 64 : (l + 1) * 64, kc * 128 : (kc + 1) * 128
                                    ],
                                    rhs=qkT_sb[qm][
                                        l * 64 : (l + 1) * 64, qt * 512 : (qt + 1) * 512
                                    ],
                                    start=True,
                                    stop=True,
                                    tile_position=(l * 64, 0),
                                )
                            return ps_s

                        # software pipeline: scores run one k-chunk ahead so the
                        # PE never sits in-order behind ctx(k)'s wait on exp(k).
                        # probs are written as fp8 in kc pairs; each pair is one
                        # DoubleRow ctx matmul (2 fp8 weights/cell, K=256).
                        ps_s = emit_scores(0)
                        pT2 = None
                        for kc in range(NTOK):
                            kc2, sl = kc // 2, kc % 2
                            if sl == 0:
                                pT2 = pT_pool.tile(
                                    [128, 2, 1024], dt.float8e4, name="pT2"
                                )
                            ps_s_next = emit_scores(kc + 1) if kc + 1 < NTOK else None
                            if kc in DVE_KC:
                                # vector-engine poly exp (frees the ACT engine)
                                nc.vector._custom_dve(
                                    EXP_OP,
                                    out=pT2[:, sl, :],
                                    in0=ps_s,
                                    s0=EXP_S0,
                                    s1=EXP_S1,
                                    imm2=EXP_IMM2,
                                )
                            else:
                                nc.scalar.activation(
                                    out=pT2[:, sl, :],
                                    in_=ps_s,
                                    func=Act.Exp,
                                    scale=0.125 / 1024,
                                )
                            ps_s = ps_s_next
                            # ctx^T (+ denominator rows 64:128): one DoubleRow
                            # matmul per kc pair per head, accumulated over kc2
                            if sl == 1:
                                for l in range(2):
                                    g = 2 * p + l
                                    nc.tensor.matmul(
                                        ctx_ps[l],
                                        lhsT=v2_sb[kc2][
                                            :, :, g * 128 : (g + 1) * 128
                                        ],
                                        rhs=pT2[:, :, l * 512 : (l + 1) * 512],
                                        start=(kc2 == 0),
                                        stop=(kc2 == NTOK // 2 - 1),
                                        perf_mode=mybir.MatmulPerfMode.DoubleRow,
                                    )
                            # first q-tile: produce the remaining V chunks just
                            # ahead of their use (ctx(kc) needs v_sb[kc]); later
                            # q-tiles: weave previous q-tiles' dense+LN stages
                            # (which consume those q-tiles' A2As) per WEAVE.
                            # The pipeline runs ~1.5 q-tiles behind attention:
                            # the early collectives are 2-3x slower than steady
                            # state, and a fetch dispatched before its A2A
                            # completes would block the in-order sync queue
                            # (delaying the next q-tile's staging DMAs).
                            if p == 0 and qt == 0 and kc + 4 < NTOK:
                                emit_v_chunk(kc + 4)
                            act = WEAVE.get((qt, p, kc))
                            if act is not None:
                                fn, dqt = act
                                fn(dqt)
                        # normalize: ctx[0:64] / den[64:128] -> ctxT (fp8);
                        # both heads' denominators share one reciprocal pass
                        # (reciprocal_approx_fast must NOT read PSUM directly:
                        # that produced NaNs; the SBUF den2 copy is load-bearing)
                        den2 = work.tile([128, 512], dt.float32, name="den2")
                        for l in range(2):
                            nc.vector.tensor_copy(
                                out=den2[l * 64 : (l + 1) * 64, :],
                                in_=ctx_ps[l][64:128, :],
                            )
                        rec = work.tile([128, 512], dt.float32, name="rec")
                        nc.vector.reciprocal_approx_fast(out=rec, in_=den2)
                        for l in range(2):
                            nc.vector.tensor_tensor(
                                out=ctxT_sb[p][
                                    l * 64 : (l + 1) * 64, qt * 512 : (qt + 1) * 512
                                ],
                                in0=ctx_ps[l][0:64, :],
                                in1=rec[l * 64 : (l + 1) * 64, :],
                                op=Alu.mult,
                            )
                        emit_a2a(qt, p)
                # last q-tile's dense+LN have no following attention to hide
                # in; qt2's LN tail fills the final exchange's flight time
                emit_ln_fin(NQT - 2)
                emit_ctx_fetch(NQT - 1)
                emit_dense(NQT - 1)
                emit_ln_stats(NQT - 1)
                emit_ln_fin(NQT - 1)

    nc.compile()
    return nc


_PROGRAM = None


def _get_program():
    global _PROGRAM
    if _PROGRAM is None:
        _PROGRAM = _build_program()
    return _PROGRAM


def _prep_core_inputs(hidden_states, w_qkv, b_qkv, w_dense, b_dense):
    """Build the 8 per-core input maps (numpy, host-side sharding)."""
    hs = np.asarray(hidden_states, dtype=np.float32)
    w_qkv = np.asarray(w_qkv, dtype=np.float32)
    b_qkv = np.asarray(b_qkv, dtype=np.float32)
    w_dense = np.asarray(w_dense, dtype=np.float32)
    b_dense = np.asarray(b_dense, dtype=np.float32)

    # v-channel bias folded into a host-side output bias:
    # b_out = b_dense + b_v_full @ w_dense   (b_v in ctx channel order)
    bv_full = np.empty((HIDDEN,), dtype=np.float64)
    for g in range(HEADS):
        bv_full[g * HD : (g + 1) * HD] = b_qkv[g * 192 + 128 : g * 192 + 192]
    # w_dense rows are already in (head, d) = g*64+d order, matching bv_full
    b_out = (
        b_dense.astype(np.float64)
        + bv_full @ w_dense.astype(np.float64)
    ).astype(np.float32)

    # shared across cores: both batches' hs^T in fp8, full dense weight
    hsT_bf = np.concatenate(
        [np.ascontiguousarray(hs[0].T), np.ascontiguousarray(hs[1].T)], axis=1
    ).astype(FP8)  # [1024, 4096]
    # x256 prescale keeps wd in fp8e4 normal range; the dense partials come
    # out x256 and the residual is prescaled to match (LN is scale-invariant)
    wd_bf = (w_dense * 256).astype(FP8)  # [1024, 1024], rows channel-ordered

    in_maps = []
    for r in range(N_CORES):
        gheads = [2 * r, 2 * r + 1]

        # wqk column order: K h0 | K h1 | Q h0 | Q h1 (64 each)
        wqk_cols = np.empty((HIDDEN, 256), dtype=np.float32)
        bqk_vec = np.empty((256,), dtype=np.float32)
        for l, g in enumerate(gheads):
            kcol = slice(g * 192 + 64, g * 192 + 128)
            qcol = slice(g * 192, g * 192 + 64)
            wqk_cols[:, l * 64 : (l + 1) * 64] = w_qkv[:, kcol]
            wqk_cols[:, 128 + l * 64 : 128 + (l + 1) * 64] = w_qkv[:, qcol]
            bqk_vec[l * 64 : (l + 1) * 64] = b_qkv[kcol]
            bqk_vec[128 + l * 64 : 128 + (l + 1) * 64] = b_qkv[qcol]

        wv_cols = np.empty((HIDDEN, 128), dtype=np.float32)
        for l, g in enumerate(gheads):
            wv_cols[:, l * 64 : (l + 1) * 64] = w_qkv[
                :, g * 192 + 128 : g * 192 + 192
            ]

        # residual shard (+ folded output bias): row qt*128 + b*64 + t
        # covers full[b, qt*512 + r*64 + t]
        res = np.empty((OUT_ROWS, HIDDEN), dtype=np.float32)
        for qt in range(NQT):
            t0 = qt * 512 + r * 64
            for b in range(B):
                res[qt * 128 + b * 64 : qt * 128 + (b + 1) * 64, :] = 256 * (
                    hs[b, t0 : t0 + 64, :] + b_out
                )

        in_maps.append(
            {
                "hsT": hsT_bf,
                "wqk": (wqk_cols * 32).astype(FP8),
                "wv": (wv_cols * 32).astype(FP8),
                "wd": wd_bf,
                "bqk": (bqk_vec * 32).reshape(256, 1),
                "hs_res": res.astype(BF16),
            }
        )
    return in_maps


def kernel(hidden_states, w_qkv, b_qkv, w_dense, b_dense, ln_gamma, ln_beta,
           _return_perf=False, **run_kwargs):
    ln_gamma = np.asarray(ln_gamma, dtype=np.float32)
    ln_beta = np.asarray(ln_beta, dtype=np.float32)
    gamma_one = np.allclose(ln_gamma, 1.0)
    beta_zero = np.allclose(ln_beta, 0.0)

    nc = _get_program()
    in_maps = _prep_core_inputs(hidden_states, w_qkv, b_qkv, w_dense, b_dense)
    res = run_bass_kernel_spmd(
        nc, in_maps, core_ids=list(range(N_CORES)), **run_kwargs
    )

    full = np.empty((B, S, HIDDEN), dtype=np.float32)
    for r in range(N_CORES):
        o = res.results[r]["out"]
        for qt in range(NQT):
            t0 = qt * 512 + r * 64
            for b in range(B):
                full[b, t0 : t0 + 64, :] = o[
                    qt * 128 + b * 64 : qt * 128 + (b + 1) * 64, :
                ]

    if not (gamma_one and beta_zero):
        # spec fills gamma=ones, beta=zeros; fall back on host if they differ
        full = full * ln_gamma[None, None, :] + ln_beta[None, None, :]

    if _return_perf:
        return full, res
    return full


# revision 32
# speedup vs baseline: 1.2307x; 1.0185x over previous
"""BERT self-attention block (QKV -> attention -> dense -> residual+LN) on 8 trn2 NeuronCores.

Sharding: tensor-parallel over heads across all 8 cores (2 heads/core), with BOTH
batch elements resident on every core (batch plays the "pair" role in the attention
weave). After each q-tile's attention, an 8-core AllToAll exchanges ctx^T (fp8e4,
128KB/chunk) so each core owns all 1024 ctx channels for its 128-token shard; the
core then runs the full dense projection (fp8 DoubleRow, weights x256 host-prescaled,
residual prescaled to match -- LayerNorm is scale-invariant) + residual + LN locally,
and the host reassembles the [2, 2048, 1024] output.

This replaced a DP(batch) x TP4(heads) scheme whose dense-partial ReduceScatter
(4MB/core, serialized ~156us CC chain) dominated the tail; the ctx A2A moves 16x
fewer bytes (fp8 + no partial duplication) and fires right after attention.

Schedule (measured 219-243us vs 297us staged baseline; run-to-run thermal
variance is +-5-8%):
- a tiny warm-up AllToAll fires at kernel start to absorb the first-collective
  trigger latency; wd/res input DMAs dispatch last (not needed until ~120us);
- per q-tile ctx staging happens right after each batch-pass's normalize; the
  last q-tile splits its exchange per batch so the tail waits only on a 64KB op;
- dense+LN for q-tile k are woven ~1.5 q-tiles behind attention (the early
  collectives run 2-3x slower than steady state, and a fetch dispatched before
  its A2A completes would head-of-line-block the in-order sync queue);
- softmax exp splits between the scalar engine (ACT spline exp) and a custom
  vector-engine op (quadratic poly + 4 squarings ~= exp(x/8)), alternating
  engines within each kc pair; probs are written fp8e4 in kc pairs, each pair
  one DoubleRow ctx matmul (K=256);
- QKV projections run fp8 DoubleRow (weights x32 host-prescaled, the x1024
  score scale folded into the exp constants, denominator 'ones' = 32.0),
  c-outer so compute starts while input DMAs are in flight; scores pairs use
  tile_position row groups and run concurrently on the PE.

Known hazards baked into this code: DVE inputs cannot shift partitions downward
and reciprocal_approx_fast cannot read PSUM (both NaN); GpSimd ops cannot read
PSUM (compile error); the CC mesh parallelizes across dim-0 rows, so cc tensors
keep many short rows.
"""

import sys

for _p in ("/opt/trn_rl_repo",):
    if _p not in sys.path:
        sys.path.insert(0, _p)

import numpy as np
import ml_dtypes

import concourse.bass as bass
import concourse.mybir as mybir
import concourse.tile as tile
from concourse import bacc
from concourse.bass_utils import run_bass_kernel_spmd

BF16 = ml_dtypes.bfloat16
FP8 = ml_dtypes.float8_e4m3

HIDDEN = 1024
HEADS = 16
HD = 64  # head dim
B = 2
S = 2048
LN_EPS = 1e-5

N_CORES = 8
LHEADS = 2  # heads per core
PAIRS = 2  # attention passes per q-tile: pair p = batch p (2 local heads each)
NCD = HIDDEN // 128  # 8 contraction chunks
NTOK = S // 128  # 16 token chunks (per batch)
NQT = 4  # attention q-tiles (512 q each)
QT = S // NQT  # 512
REPLICA_GROUPS = [[0, 1, 2, 3, 4, 5, 6, 7]]
# per-core output: for each q-tile, 64 tokens of each batch
# (rows qt*128 + b*64 + t  <->  full[b, qt*512 + rank*64 + t])
OUT_ROWS = NQT * 128  # 512

# which kc chunks the vector engine handles for exp (rest go to ACT)
DVE_KC = frozenset((1, 5, 7, 11, 13))

dt = mybir.dt
Alu = mybir.AluOpType
Act = mybir.ActivationFunctionType

# ---------------- custom DVE op: poly exp ----------------
# out = (imm2 + x*(s0 + x*s1))^16  ~=  exp(x/8) for x in +-28 (raw q.k scores)
# (quadratic fit of exp(t) on t = x/128 in +-0.225, then 4 squarings)
_CQ = (1.00004518, 1.00351622, 0.49634025)
EXP_S0 = float(_CQ[1] / 128 / 1024)
EXP_S1 = float(_CQ[2] / (128 * 128) / (1024 * 1024))
EXP_IMM2 = float(_CQ[0])


def _register_exp_op():
    from concourse import dve_ops as DO
    from concourse.dve_spec import Spec, Src0, C0, C1, C2, lower
    from concourse.dve_spec import _has_src1 as has_src1
    from concourse.dve_uop import DveOpSpec

    name = "EXP_Q4_ANT"
    for o in DO.OPS:
        if o.name == name:
            return o
    a1 = Src0 * C1 + C0
    a2 = Src0 * a1 + C2
    p2 = a2 * a2
    p4 = p2 * p2
    p8 = p4 * p4
    body = p8 * p8

    def _ref(in0, in1, s0, s1, imm2):
        p = imm2 + in0 * (s0 + in0 * s1)
        for _ in range(4):
            p = p * p
        return p

    spec = Spec(body=body, reference=_ref)
    row = DO._CUSTOM_DVE_ROW_BASE + len(DO.OPS)
    DO._SUB_OPCODE_FOR_NAME[name] = row
    shas = {}
    for ver in ("v3", "v4"):
        uops = lower(spec, ver=ver)
        shas[ver] = DveOpSpec(
            name=name, opcode=row, uops=uops, rd1_en=has_src1(spec)
        ).sha(ver)
    op = DO.DveOp(name, spec, subdim=False, uops_sha=shas)
    DO.OPS.append(op)
    DO.CUSTOM_DVE_SPECS[name] = spec
    return op


EXP_OP = _register_exp_op()


def _build_program():
    nc = bacc.Bacc(
        "TRN2", target_bir_lowering=False, debug=False, num_devices=N_CORES
    )

    # Route Exp and Ln to the one table set that holds both, so the kernel
    # never reloads ACT tables (set ids are positional; only values change).
    from concourse import hw_specs

    for name, funcs in hw_specs.get_activation_tables(nc.m.arch).items():
        if name != "natural_log_exp_and_others":
            funcs.discard(Act.Exp)
            funcs.discard(Act.Ln)

    # ---------------- DRAM I/O ----------------
    # hsT: both batches, [1024, 4096] = [hid, b*2048 + t]
    hsT = nc.dram_tensor("hsT", [HIDDEN, B * S], dt.float8e4, kind="ExternalInput")
    # wqk: [1024, 256] = [K h0 | K h1 | Q h0 | Q h1] (x32 prescale)
    wqk = nc.dram_tensor("wqk", [HIDDEN, 256], dt.float8e4, kind="ExternalInput")
    # wv: [1024, 128] = [V h0 | V h1] (x32 prescale)
    wv = nc.dram_tensor("wv", [HIDDEN, 128], dt.float8e4, kind="ExternalInput")
    # wd: full dense weight [1024, 1024]
    wd = nc.dram_tensor("wd", [HIDDEN, HIDDEN], dt.float8e4, kind="ExternalInput")
    bqk = nc.dram_tensor("bqk", [256, 1], dt.float32, kind="ExternalInput")
    # residual (+ folded dense bias) for this core's token shard
    hs_res = nc.dram_tensor(
        "hs_res", [OUT_ROWS, HIDDEN], dt.bfloat16, kind="ExternalInput"
    )
    out = nc.dram_tensor("out", [OUT_ROWS, HIDDEN], dt.float32, kind="ExternalOutput")

    # internal DRAM for the collective (cannot use I/O tensors)
    # cc layout per qt: [8 peer blocks * 128 chan, 128] where block r =
    # my 128 channels for tokens qt*512 + r*64 (+64 of each batch:
    # cols 0:64 = batch0, 64:128 = batch1). NOTE: a [64, 2048] variant
    # (4KB rows) measured SLOWER -- the CC mesh parallelizes across rows,
    # so keep many rows.
    # qt0-2 exchange once per q-tile (fewer sync-queue wait points in the
    # steady state); the LAST q-tile splits per batch so the tail only
    # waits on a 64KB op and batch0's half hides under the p==1 pass
    cc_in = [
        [nc.dram_tensor(f"cc_in{q}", [N_CORES * 128, 128], dt.float8e4)]
        if q < NQT - 1
        else [
            nc.dram_tensor(f"cc_in{q}_{b}", [N_CORES * 128, 64], dt.float8e4)
            for b in range(2)
        ]
        for q in range(NQT)
    ]
    cc_out = [
        [nc.dram_tensor(f"cc_out{q}", [N_CORES * 128, 128], dt.float8e4)]
        if q < NQT - 1
        else [
            nc.dram_tensor(f"cc_out{q}_{b}", [N_CORES * 128, 64], dt.float8e4)
            for b in range(2)
        ]
        for q in range(NQT)
    ]
    # tiny dummy exchange fired at kernel start: absorbs the ~11.5us
    # first-collective trigger delay + CC DMA-ring spin-up so A2A(qt0)
    # runs at warm-stream speed
    cc_warm_in = nc.dram_tensor("cc_warm_in", [N_CORES, 128], dt.bfloat16)
    cc_warm_out = nc.dram_tensor("cc_warm_out", [N_CORES, 128], dt.bfloat16)

    with tile.TileContext(nc) as tc:
        with (
            tc.tile_pool(name="persist", bufs=1) as persist,
            tc.tile_pool(name="pT_pool", bufs=6) as pT_pool,
            tc.tile_pool(name="work", bufs=3) as work,
            tc.tile_pool(name="ln", bufs=2) as lnp,
        ):
            # ---------------- persistent SBUF loads ----------------
            zero_sb = persist.tile([128, 1], dt.float32, name="zero_sb")
            nc.vector.memset(zero_sb, 0.0)
            nc.const_aps.aps[(dt.float32, 0.0)] = zero_sb
            eps_sb = persist.tile([128, 1], dt.float32, name="eps_sb")
            nc.vector.memset(eps_sb, LN_EPS)
            # warm the CC stream before any data dep can delay the trigger
            nc.gpsimd.collective_compute(
                "AllToAll",
                Alu.bypass,
                replica_groups=REPLICA_GROUPS,
                ins=[cc_warm_in[:, :].opt()],
                outs=[cc_warm_out[:, :].opt()],
            )
            # input DMAs: interleaved so the c-outer QK matmuls can start
            # after the first hsT/wqk chunk pair lands (the sync queue
            # serializes at ~0.6us per dma_start dispatch, so keep them few).
            # wd/res (3MB) aren't consumed until ~120us in: dispatch them
            # last so they don't steal HBM bandwidth from the hsT stream.
            hsT_all = persist.tile([128, NCD, B * S], dt.float8e4, name="hsT_all")
            hsT_r = hsT[:, :].rearrange("(c p) t -> p c t", p=128)
            wqk_all = persist.tile([128, NCD, 256], dt.float8e4, name="wqk_all")
            wqk_r = wqk[:, :].rearrange("(c p) n -> p c n", p=128)
            nc.sync.dma_start(out=wqk_all[:, 0:8, :], in_=wqk_r[:, 0:8, :])
            nc.sync.dma_start(out=hsT_all[:, 0:2, :], in_=hsT_r[:, 0:2, :])
            nc.sync.dma_start(out=hsT_all[:, 2:4, :], in_=hsT_r[:, 2:4, :])
            nc.sync.dma_start(out=hsT_all[:, 4:6, :], in_=hsT_r[:, 4:6, :])
            nc.sync.dma_start(out=hsT_all[:, 6:8, :], in_=hsT_r[:, 6:8, :])
            bqk_all = persist.tile([128, 2], dt.float32, name="bqk_all")
            nc.sync.dma_start(
                out=bqk_all, in_=bqk[:, :].rearrange("(m p) o -> p (m o)", p=128)
            )
            wv_all = persist.tile([128, NCD, 128], dt.float8e4, name="wv_all")
            nc.sync.dma_start(
                out=wv_all, in_=wv[:, :].rearrange("(c p) n -> p c n", p=128)
            )
            wd_all = persist.tile([128, NCD, HIDDEN], dt.float8e4, name="wd_all")
            nc.sync.dma_start(
                out=wd_all, in_=wd[:, :].rearrange("(c p) n -> p c n", p=128)
            )
            res_all = persist.tile([128, NQT, HIDDEN], dt.bfloat16, name="res_all")
            nc.sync.dma_start(
                out=res_all,
                in_=hs_res[:, :].rearrange("(g p) n -> p g n", p=128),
            )
            bqk_sb = [bqk_all[:, m : m + 1] for m in range(2)]

            # qkT m-chunk layout: 0=K batch0, 1=Q batch0, 2=K batch1, 3=Q batch1
            # (partitions 0:64 = local head 0, 64:128 = local head 1)
            qkT_sb = [
                persist.tile([128, S], dt.bfloat16, name=f"qkT{m}") for m in range(4)
            ]
            # V tiles (fp8, DoubleRow pairs): tile t2 slot s covers token
            # chunk 2*t2+s as 4 groups (g = 2*batch + head) of [V_h(64) | ones(64)]
            v2_sb = [
                persist.tile([128, 2, 512], dt.float8e4, name=f"v{t2}")
                for t2 in range(NTOK // 2)
            ]
            # the denominator 'ones' (=32, matching the x32 wv prescale) never
            # change: write them all here while the vector engine is idle
            # instead of inside the qt0 attention weave
            for t2 in range(NTOK // 2):
                vt_all = v2_sb[t2].rearrange("p s (g c) -> p (s g) c", c=128)
                nc.vector.memset(vt_all[:, :, 64:128], 32.0)
            # ctx^T (normalized, bf16): chunk p = batch p, partitions 0:64 =
            # local head 0, 64:128 = local head 1, cols = batch p's tokens
            ctxT_sb = [
                persist.tile([128, S], dt.float8e4, name=f"ctxT{p}")
                for p in range(PAIRS)
            ]

            # ---------------- QK projection (c-outer, all 8 PSUM banks) -------
            # qk_ps region idx = dm*4 + nh*2 + j accumulates over c; iteration c
            # only needs hsT chunk c + wqk chunk c, so compute starts while the
            # rest of the inputs are still in flight. m-chunk m: batch m//2,
            # K/Q = m%2 (wqk cols (m%2)*128).
            with tc.tile_pool(name="psqk", bufs=1, space="PSUM") as psqk:
                for mp in range(2):  # m-pass: m in {2mp, 2mp+1} = batch mp
                    qk_ps = psqk.tile([128, 8, 512], dt.float32, name="qk_ps")
                    for cp in range(NCD // 2):
                        for dm in range(2):
                            m = 2 * mp + dm
                            for nh in range(2):
                                for j in range(2):
                                    nc.tensor.matmul(
                                        qk_ps[:, dm * 4 + nh * 2 + j, :],
                                        lhsT=wqk_all[
                                            :,
                                            2 * cp : 2 * cp + 2,
                                            dm * 128 : (dm + 1) * 128,
                                        ],
                                        rhs=hsT_all[
                                            :,
                                            2 * cp : 2 * cp + 2,
                                            mp * 2048
                                            + nh * 1024
                                            + j * 512 : mp * 2048
                                            + nh * 1024
                                            + (j + 1) * 512,
                                        ],
                                        start=(cp == 0),
                                        stop=(cp == NCD // 2 - 1),
                                        perf_mode=mybir.MatmulPerfMode.DoubleRow,
                                    )
                    for dm in range(2):
                        m = 2 * mp + dm
                        for nh in range(2):
                            nc.scalar.activation(
                                out=qkT_sb[m][:, nh * 1024 : (nh + 1) * 1024],
                                in_=qk_ps[
                                    :, dm * 4 + nh * 2 : dm * 4 + nh * 2 + 2, :
                                ],
                                func=Act.Identity,
                                bias=bqk_sb[dm],
                            )

            # psqk released; attention pools take over PSUM
            with (
                tc.tile_pool(name="psmm", bufs=3, space="PSUM") as psmm,
                tc.tile_pool(name="psctx", bufs=1, space="PSUM") as psctx,
            ):
                # V[tc] group g=2b+l: cols l*... ps[:, b*128+l*64 : +64] =
                # hs[b, tok_chunk] @ wv[:, l*64:...]; v tile cols g*128+64 :
                # (g+1)*128 are constant 32.0 (denominator trick)
                def emit_v_chunk(t):
                    ps = psmm.tile([128, 1024], dt.float32, name="ps_mm")
                    for b in range(2):
                        for cp in range(NCD // 2):
                            nc.tensor.matmul(
                                ps[:, b * 128 : (b + 1) * 128],
                                lhsT=hsT_all[
                                    :,
                                    2 * cp : 2 * cp + 2,
                                    b * 2048 + t * 128 : b * 2048 + (t + 1) * 128,
                                ],
                                rhs=wv_all[:, 2 * cp : 2 * cp + 2, :],
                                start=(cp == 0),
                                stop=(cp == NCD // 2 - 1),
                                perf_mode=mybir.MatmulPerfMode.DoubleRow,
                            )
                    vt = v2_sb[t // 2][:, t % 2, :].rearrange(
                        "p (g c) -> p g c", c=128
                    )
                    # alternate the evacuation engine so qt0's V weave doesn't
                    # pile 12 copies onto the vector engine alone
                    if t % 2 == 0:
                        nc.scalar.activation(
                            out=vt[:, :, 0:64],
                            in_=ps[:, 0:256].rearrange("p (g c) -> p g c", c=64),
                            func=Act.Identity,
                        )
                    else:
                        nc.vector.tensor_copy(
                            out=vt[:, :, 0:64],
                            in_=ps[:, 0:256].rearrange("p (g c) -> p g c", c=64),
                        )

                for t in range(4):
                    emit_v_chunk(t)

                # ------------- phase 2: attention + A2A + dense + LN ----------
                # q-tile-major; after each q-tile's ctx is normalized, the
                # chunk's ctxT slices are DMAed out and an 8-core AllToAll
                # fires. dense+LN for qt-1 are woven into qt's second (p==1)
                # attention pass, by which point A2A(qt-1) has long landed.
                def emit_a2a(qt, b):
                    # batch b's ctxT slice is final right after pass p=b's
                    # normalize: stage it immediately; trigger per-half for
                    # the last q-tile, once per q-tile otherwise
                    if qt < NQT - 1:
                        cin = cc_in[qt][0][:, :].rearrange(
                            "(r c) (bb t) -> c r bb t", c=128, t=64
                        )[:, :, b, :]
                    else:
                        cin = cc_in[qt][b][:, :].rearrange(
                            "(r c) t -> c r t", c=128
                        )
                    nc.sync.dma_start(
                        out=cin,
                        in_=ctxT_sb[b][
                            :, qt * 512 : (qt + 1) * 512
                        ].rearrange("c (r t) -> c r t", t=64),
                    )
                    if qt == NQT - 1 or b == 1:
                        idx = b if qt == NQT - 1 else 0
                        nc.gpsimd.collective_compute(
                            "AllToAll",
                            Alu.bypass,
                            replica_groups=REPLICA_GROUPS,
                            ins=[cc_in[qt][idx][:, :].opt()],
                            outs=[cc_out[qt][idx][:, :].opt()],
                        )

                # dense + residual + LN for one q-tile's 128-token shard,
                # staged so each piece slots into engine slack of the covering
                # attention pass (fetch / matmul blob+evac / stats / finish)
                dense_state = {}

                def emit_ctx_fetch(qt):
                    ctx_sb = work.tile([128, NCD, 128], dt.float8e4, name="ctx_sb")
                    if qt < NQT - 1:
                        nc.sync.dma_start(
                            out=ctx_sb,
                            in_=cc_out[qt][0][:, :].rearrange(
                                "(c p) t -> p c t", p=128
                            ),
                        )
                    else:
                        for b in range(2):
                            nc.sync.dma_start(
                                out=ctx_sb[:, :, b * 64 : (b + 1) * 64],
                                in_=cc_out[qt][b][:, :].rearrange(
                                    "(c p) t -> p c t", p=128
                                ),
                            )
                    dense_state["ctx_sb"] = ctx_sb

                def emit_dense(qt):
                    # 16 matmuls + immediate add-evacuation (x = dense + res).
                    # ps_d's full lifetime is inside this call, so sharing the
                    # ps_mm rotation with the scores pipeline is safe.
                    ctx_sb = dense_state["ctx_sb"]
                    ps_d = psmm.tile([128, 1024], dt.float32, name="ps_mm")
                    for cp in range(NCD // 2):
                        for j in range(2):
                            nc.tensor.matmul(
                                ps_d[:, j * 512 : (j + 1) * 512],
                                lhsT=ctx_sb[:, 2 * cp : 2 * cp + 2, :],
                                rhs=wd_all[:, 2 * cp : 2 * cp + 2, j * 512 : (j + 1) * 512],
                                start=(cp == 0),
                                stop=(cp == NCD // 2 - 1),
                                perf_mode=mybir.MatmulPerfMode.DoubleRow,
                            )
                    x = lnp.tile([128, HIDDEN], dt.float32, name="x")
                    nc.vector.tensor_tensor(
                        out=x, in0=ps_d, in1=res_all[:, qt, :], op=Alu.add
                    )
                    dense_state["x"] = x

                def emit_ln_stats(qt):
                    x = dense_state["x"]
                    stats = lnp.tile([128, 2, 6], dt.float32, name="stats")
                    xv = x.rearrange("p (s f) -> p s f", f=512)
                    for i in range(2):
                        nc.vector.bn_stats(out=stats[:, i, :], in_=xv[:, i, :])
                    mv = lnp.tile([128, 2], dt.float32, name="mv")
                    nc.vector.bn_aggr(out=mv, in_=stats)
                    dense_state["mv"] = mv

                def emit_ln_fin(qt):
                    x = dense_state["x"]
                    mv = dense_state["mv"]
                    # rstd = exp(-0.5 * ln(var + eps)) -- stays in the exp/ln tables
                    lnv = lnp.tile([128, 1], dt.float32, name="lnv")
                    nc.scalar.activation(
                        out=lnv, in_=mv[:, 1:2], func=Act.Ln, bias=eps_sb
                    )
                    rstd = lnp.tile([128, 1], dt.float32, name="rstd")
                    nc.scalar.activation(
                        out=rstd, in_=lnv, func=Act.Exp, scale=-0.5
                    )
                    y = lnp.tile([128, HIDDEN], dt.float32, name="y")
                    nc.vector.tensor_scalar(
                        out=y,
                        in0=x,
                        scalar1=mv[:, 0:1],
                        scalar2=rstd,
                        op0=Alu.subtract,
                        op1=Alu.mult,
                    )
                    nc.sync.dma_start(
                        out=out[qt * 128 : (qt + 1) * 128, :], in_=y
                    )

                WEAVE = {
                    (2, 0, 10): (emit_ctx_fetch, 0),
                    (2, 0, 14): (emit_dense, 0),
                    (2, 1, 4): (emit_ln_stats, 0),
                    (2, 1, 8): (emit_ln_fin, 0),
                    (2, 1, 14): (emit_ctx_fetch, 1),
                    (3, 0, 6): (emit_dense, 1),
                    (3, 0, 10): (emit_ln_stats, 1),
                    (3, 0, 14): (emit_ln_fin, 1),
                    (3, 1, 8): (emit_ctx_fetch, 2),
                    (3, 1, 12): (emit_dense, 2),
                }

                for qt in range(NQT):
                    for p in range(PAIRS):
                        km = 2 * p  # K m-chunk (batch p)
                        qm = 2 * p + 1  # Q m-chunk (batch p)
                        ctx_ps = [
                            psctx.tile([128, 512], dt.float32, name=f"ps_ctx{l}")
                            for l in range(2)
                        ]

                        def emit_scores(kc, km=km, qm=qm, qt=qt):
                            ps_s = psmm.tile([128, 1024], dt.float32, name="ps_mm")
                            # scores^T for both local heads (concurrent row
                            # groups: head0 rows 0:64, head1 rows 64:128)
                            for l in range(2):
                                nc.tensor.matmul(
                                    ps_s[:, l * 512 : (l + 1) * 512],
                                    lhsT=qkT_sb[km][
                                        l * 64 : (l + 1) * 64, kc * 128 : (kc + 1) * 128
                                    ],
                                    rhs=qkT_sb[qm][
                                        l * 64 : (l + 1) * 64, qt * 512 : (qt + 1) * 512
                                    ],
                                    start=True,
                                    stop=True,
                                    tile_position=(l * 64, 0),
                                )
                            return ps_s

                        # software pipeline: scores run one k-chunk ahead so the
                        # PE never sits in-order behind ctx(k)'s wait on exp(k).
                        # probs are written as fp8 in kc pairs; each pair is one
                        # DoubleRow ctx matmul (2 fp8 weights/cell, K=256).
                        ps_s = emit_scores(0)
                        pT2 = None
                        for kc in range(NTOK):
                            kc2, sl = kc // 2, kc % 2
                            if sl == 0:
                                pT2 = pT_pool.tile(
                                    [128, 2, 1024], dt.float8e4, name="pT2"
                                )
                            ps_s_next = emit_scores(kc + 1) if kc + 1 < NTOK else None
                            if kc in DVE_KC:
                                # vector-engine poly exp (frees the ACT engine)
                                nc.vector._custom_dve(
                                    EXP_OP,
                                    out=pT2[:, sl, :],
                                    in0=ps_s,
                                    s0=EXP_S0,
                                    s1=EXP_S1,
                                    imm2=EXP_IMM2,
                                )
                            else:
                                nc.scalar.activation(
                                    out=pT2[:, sl, :],
                                    in_=ps_s,
                                    func=Act.Exp,
                                    scale=0.125 / 1024,
                                )
                            ps_s = ps_s_next
                            # ctx^T (+ denominator rows 64:128): one DoubleRow
                            # matmul per kc pair per head, accumulated over kc2
                            if sl == 1:
                                for l in range(2):
                                    g = 2 * p + l
                                    nc.tensor.matmul(
                                        ctx_ps[l],
                                        lhsT=v2_sb[kc2][
                                            :, :, g * 128 : (g + 1) * 128
                                        ],
                                        rhs=pT2[:, :, l * 512 : (l + 1) * 512],
                                        start=(kc2 == 0),
                                        stop=(kc2 == NTOK // 2 - 1),
                                        perf_mode=mybir.MatmulPerfMode.DoubleRow,
                                    )
                            # first q-tile: produce the remaining V chunks just
                            # ahead of their use (ctx(kc) needs v_sb[kc]); later
                            # q-tiles: weave previous q-tiles' dense+LN stages
                            # (which consume those q-tiles' A2As) per WEAVE.
                            # The pipeline runs ~1.5 q-tiles behind attention:
                            # the early collectives are 2-3x slower than steady
                            # state, and a fetch dispatched before its A2A
                            # completes would block the in-order sync queue
                            # (delaying the next q-tile's staging DMAs).
                            if p == 0 and qt == 0 and kc + 4 < NTOK:
                                emit_v_chunk(kc + 4)
                            act = WEAVE.get((qt, p, kc))
                            if act is not None:
                                fn, dqt = act
                                fn(dqt)
                        # normalize: ctx[0:64] / den[64:128] -> ctxT (fp8);
                        # both heads' denominators share one reciprocal pass
                        # (reciprocal_approx_fast must NOT read PSUM directly:
                        # that produced NaNs; the SBUF den2 copy is load-bearing)
                        den2 = work.tile([128, 512], dt.float32, name="den2")
                        for l in range(2):
                            nc.vector.tensor_copy(
                                out=den2[l * 64 : (l + 1) * 64, :],
                                in_=ctx_ps[l][64:128, :],
                            )
                        rec = work.tile([128, 512], dt.float32, name="rec")
                        nc.vector.reciprocal_approx_fast(out=rec, in_=den2)
                        for l in range(2):
                            nc.vector.tensor_tensor(
                                out=ctxT_sb[p][
                                    l * 64 : (l + 1) * 64, qt * 512 : (qt + 1) * 512
                                ],
                                in0=ctx_ps[l][0:64, :],
                                in1=rec[l * 64 : (l + 1) * 64, :],
                                op=Alu.mult,
                            )
                        emit_a2a(qt, p)
                # last q-tile's dense+LN have no following attention to hide
                # in; qt2's LN tail fills the final exchange's flight time
                emit_ln_stats(NQT - 2)
                emit_ln_fin(NQT - 2)
                emit_ctx_fetch(NQT - 1)
                emit_dense(NQT - 1)
                emit_ln_stats(NQT - 1)
                emit_ln_fin(NQT - 1)

    nc.compile()
    return nc


_PROGRAM = None


def _get_program():
    global _PROGRAM
    if _PROGRAM is None:
        _PROGRAM = _build_program()
    return _PROGRAM


def _prep_core_inputs(hidden_states, w_qkv, b_qkv, w_dense, b_dense):
    """Build the 8 per-core input maps (numpy, host-side sharding)."""
    hs = np.asarray(hidden_states, dtype=np.float32)
    w_qkv = np.asarray(w_qkv, dtype=np.float32)
    b_qkv = np.asarray(b_qkv, dtype=np.float32)
    w_dense = np.asarray(w_dense, dtype=np.float32)
    b_dense = np.asarray(b_dense, dtype=np.float32)

    # v-channel bias folded into a host-side output bias:
    # b_out = b_dense + b_v_full @ w_dense   (b_v in ctx channel order)
    bv_full = np.empty((HIDDEN,), dtype=np.float64)
    for g in range(HEADS):
        bv_full[g * HD : (g + 1) * HD] = b_qkv[g * 192 + 128 : g * 192 + 192]
    # w_dense rows are already in (head, d) = g*64+d order, matching bv_full
    b_out = (
        b_dense.astype(np.float64)
        + bv_full @ w_dense.astype(np.float64)
    ).astype(np.float32)

    # shared across cores: both batches' hs^T in fp8, full dense weight
    hsT_bf = np.concatenate(
        [np.ascontiguousarray(hs[0].T), np.ascontiguousarray(hs[1].T)], axis=1
    ).astype(FP8)  # [1024, 4096]
    # x256 prescale keeps wd in fp8e4 normal range; the dense partials come
    # out x256 and the residual is prescaled to match (LN is scale-invariant)
    wd_bf = (w_dense * 256).astype(FP8)  # [1024, 1024], rows channel-ordered

    in_maps = []
    for r in range(N_CORES):
        gheads = [2 * r, 2 * r + 1]

        # wqk column order: K h0 | K h1 | Q h0 | Q h1 (64 each)
        wqk_cols = np.empty((HIDDEN, 256), dtype=np.float32)
        bqk_vec = np.empty((256,), dtype=np.float32)
        for l, g in enumerate(gheads):
            kcol = slice(g * 192 + 64, g * 192 + 128)
            qcol = slice(g * 192, g * 192 + 64)
            wqk_cols[:, l * 64 : (l + 1) * 64] = w_qkv[:, kcol]
            wqk_cols[:, 128 + l * 64 : 128 + (l + 1) * 64] = w_qkv[:, qcol]
            bqk_vec[l * 64 : (l + 1) * 64] = b_qkv[kcol]
            bqk_vec[128 + l * 64 : 128 + (l + 1) * 64] = b_qkv[qcol]

        wv_cols = np.empty((HIDDEN, 128), dtype=np.float32)
        for l, g in enumerate(gheads):
            wv_cols[:, l * 64 : (l + 1) * 64] = w_qkv[
                :, g * 192 + 128 : g * 192 + 192
            ]

        # residual shard (+ folded output bias): row qt*128 + b*64 + t
        # covers full[b, qt*512 + r*64 + t]
        res = np.empty((OUT_ROWS, HIDDEN), dtype=np.float32)
        for qt in range(NQT):
            t0 = qt * 512 + r * 64
            for b in range(B):
                res[qt * 128 + b * 64 : qt * 128 + (b + 1) * 64, :] = 256 * (
                    hs[b, t0 : t0 + 64, :] + b_out
                )

        in_maps.append(
            {
                "hsT": hsT_bf,
                "wqk": (wqk_cols * 32).astype(FP8),
                "wv": (wv_cols * 32).astype(FP8),
                "wd": wd_bf,
                "bqk": (bqk_vec * 32).reshape(256, 1),
                "hs_res": res.astype(BF16),
            }
        )
    return in_maps


def kernel(hidden_states, w_qkv, b_qkv, w_dense, b_dense, ln_gamma, ln_beta,
           _return_perf=False, **run_kwargs):
    ln_gamma = np.asarray(ln_gamma, dtype=np.float32)
    ln_beta = np.asarray(ln_beta, dtype=np.float32)
    gamma_one = np.allclose(ln_gamma, 1.0)
    beta_zero = np.allclose(ln_beta, 0.0)

    nc = _get_program()
    in_maps = _prep_core_inputs(hidden_states, w_qkv, b_qkv, w_dense, b_dense)
    res = run_bass_kernel_spmd(
        nc, in_maps, core_ids=list(range(N_CORES)), **run_kwargs
    )

    full = np.empty((B, S, HIDDEN), dtype=np.float32)
    for r in range(N_CORES):
        o = res.results[r]["out"]
        for qt in range(NQT):
            t0 = qt * 512 + r * 64
            for b in range(B):
                full[b, t0 : t0 + 64, :] = o[
                    qt * 128 + b * 64 : qt * 128 + (b + 1) * 64, :
                ]

    if not (gamma_one and beta_zero):
        # spec fills gamma=ones, beta=zeros; fall back on host if they differ
        full = full * ln_gamma[None, None, :] + ln_beta[None, None, :]

    if _return_perf:
        return full, res
    return full


# revision 33
# speedup vs baseline: 1.2597x; 1.0235x over previous
"""BERT self-attention block (QKV -> attention -> dense -> residual+LN) on 8 trn2 NeuronCores.

Sharding: tensor-parallel over heads across all 8 cores (2 heads/core), with BOTH
batch elements resident on every core (batch plays the "pair" role in the attention
weave). After each q-tile's attention, an 8-core AllToAll exchanges ctx^T (fp8e4,
128KB/chunk) so each core owns all 1024 ctx channels for its 128-token shard; the
core then runs the full dense projection (fp8 DoubleRow, weights x256 host-prescaled,
residual prescaled to match -- LayerNorm is scale-invariant) + residual + LN locally,
and the host reassembles the [2, 2048, 1024] output.

This replaced a DP(batch) x TP4(heads) scheme whose dense-partial ReduceScatter
(4MB/core, serialized ~156us CC chain) dominated the tail; the ctx A2A moves 16x
fewer bytes (fp8 + no partial duplication) and fires right after attention.

Schedule (measured 219-243us vs 297us staged baseline; run-to-run thermal
variance is +-5-8%):
- a tiny warm-up AllToAll fires at kernel start to absorb the first-collective
  trigger latency; wd/res input DMAs dispatch last (not needed until ~120us);
- per q-tile ctx staging happens right after each batch-pass's normalize; the
  last q-tile splits its exchange per batch so the tail waits only on a 64KB op;
- dense+LN for q-tile k are woven ~1.5 q-tiles behind attention (the early
  collectives run 2-3x slower than steady state, and a fetch dispatched before
  its A2A completes would head-of-line-block the in-order sync queue);
- softmax exp splits between the scalar engine (ACT spline exp) and a custom
  vector-engine op (quadratic poly + 4 squarings ~= exp(x/8)), alternating
  engines within each kc pair; probs are written fp8e4 in kc pairs, each pair
  one DoubleRow ctx matmul (K=256);
- QKV projections run fp8 DoubleRow (weights x32 host-prescaled, the x1024
  score scale folded into the exp constants, denominator 'ones' = 32.0),
  c-outer so compute starts while input DMAs are in flight; scores pairs use
  tile_position row groups and run concurrently on the PE.

Known hazards baked into this code: DVE inputs cannot shift partitions downward
and reciprocal_approx_fast cannot read PSUM (both NaN); GpSimd ops cannot read
PSUM (compile error); the CC mesh parallelizes across dim-0 rows, so cc tensors
keep many short rows.
"""

import sys

for _p in ("/opt/trn_rl_repo",):
    if _p not in sys.path:
        sys.path.insert(0, _p)

import numpy as np
import ml_dtypes

import concourse.bass as bass
import concourse.mybir as mybir
import concourse.tile as tile
from concourse import bacc
from concourse.bass_utils import run_bass_kernel_spmd

BF16 = ml_dtypes.bfloat16
FP8 = ml_dtypes.float8_e4m3

HIDDEN = 1024
HEADS = 16
HD = 64  # head dim
B = 2
S = 2048
LN_EPS = 1e-5

N_CORES = 8
LHEADS = 2  # heads per core
PAIRS = 2  # attention passes per q-tile: pair p = batch p (2 local heads each)
NCD = HIDDEN // 128  # 8 contraction chunks
NTOK = S // 128  # 16 token chunks (per batch)
NQT = 4  # attention q-tiles (512 q each)
QT = S // NQT  # 512
REPLICA_GROUPS = [[0, 1, 2, 3, 4, 5, 6, 7]]
# per-core output: for each q-tile, 64 tokens of each batch
# (rows qt*128 + b*64 + t  <->  full[b, qt*512 + rank*64 + t])
OUT_ROWS = NQT * 128  # 512

# which kc chunks the vector engine handles for exp (rest go to ACT)
DVE_KC = frozenset((1, 5, 7, 11, 13))

dt = mybir.dt
Alu = mybir.AluOpType
Act = mybir.ActivationFunctionType

# ---------------- custom DVE op: poly exp ----------------
# out = (imm2 + x*(s0 + x*s1))^16  ~=  exp(x/8) for x in +-28 (raw q.k scores)
# (quadratic fit of exp(t) on t = x/128 in +-0.225, then 4 squarings)
_CQ = (1.00004518, 1.00351622, 0.49634025)
EXP_S0 = float(_CQ[1] / 128 / 1024)
EXP_S1 = float(_CQ[2] / (128 * 128) / (1024 * 1024))
EXP_IMM2 = float(_CQ[0])


def _register_exp_op():
    from concourse import dve_ops as DO
    from concourse.dve_spec import Spec, Src0, C0, C1, C2, lower
    from concourse.dve_spec import _has_src1 as has_src1
    from concourse.dve_uop import DveOpSpec

    name = "EXP_Q4_ANT"
    for o in DO.OPS:
        if o.name == name:
            return o
    a1 = Src0 * C1 + C0
    a2 = Src0 * a1 + C2
    p2 = a2 * a2
    p4 = p2 * p2
    p8 = p4 * p4
    body = p8 * p8

    def _ref(in0, in1, s0, s1, imm2):
        p = imm2 + in0 * (s0 + in0 * s1)
        for _ in range(4):
            p = p * p
        return p

    spec = Spec(body=body, reference=_ref)
    row = DO._CUSTOM_DVE_ROW_BASE + len(DO.OPS)
    DO._SUB_OPCODE_FOR_NAME[name] = row
    shas = {}
    for ver in ("v3", "v4"):
        uops = lower(spec, ver=ver)
        shas[ver] = DveOpSpec(
            name=name, opcode=row, uops=uops, rd1_en=has_src1(spec)
        ).sha(ver)
    op = DO.DveOp(name, spec, subdim=False, uops_sha=shas)
    DO.OPS.append(op)
    DO.CUSTOM_DVE_SPECS[name] = spec
    return op


EXP_OP = _register_exp_op()


def _build_program():
    nc = bacc.Bacc(
        "TRN2", target_bir_lowering=False, debug=False, num_devices=N_CORES
    )

    # Route Exp and Ln to the one table set that holds both, so the kernel
    # never reloads ACT tables (set ids are positional; only values change).
    from concourse import hw_specs

    for name, funcs in hw_specs.get_activation_tables(nc.m.arch).items():
        if name != "natural_log_exp_and_others":
            funcs.discard(Act.Exp)
            funcs.discard(Act.Ln)

    # ---------------- DRAM I/O ----------------
    # hsT: both batches, [1024, 4096] = [hid, b*2048 + t]
    hsT = nc.dram_tensor("hsT", [HIDDEN, B * S], dt.float8e4, kind="ExternalInput")
    # wqk: [1024, 256] = [K h0 | K h1 | Q h0 | Q h1] (x32 prescale)
    wqk = nc.dram_tensor("wqk", [HIDDEN, 256], dt.float8e4, kind="ExternalInput")
    # wv: [1024, 128] = [V h0 | V h1] (x32 prescale)
    wv = nc.dram_tensor("wv", [HIDDEN, 128], dt.float8e4, kind="ExternalInput")
    # wd: full dense weight [1024, 1024]
    wd = nc.dram_tensor("wd", [HIDDEN, HIDDEN], dt.float8e4, kind="ExternalInput")
    bqk = nc.dram_tensor("bqk", [256, 1], dt.float32, kind="ExternalInput")
    # residual (+ folded dense bias) for this core's token shard
    hs_res = nc.dram_tensor(
        "hs_res", [OUT_ROWS, HIDDEN], dt.bfloat16, kind="ExternalInput"
    )
    out = nc.dram_tensor("out", [OUT_ROWS, HIDDEN], dt.float32, kind="ExternalOutput")

    # internal DRAM for the collective (cannot use I/O tensors)
    # cc layout per qt: [8 peer blocks * 128 chan, 128] where block r =
    # my 128 channels for tokens qt*512 + r*64 (+64 of each batch:
    # cols 0:64 = batch0, 64:128 = batch1). NOTE: a [64, 2048] variant
    # (4KB rows) measured SLOWER -- the CC mesh parallelizes across rows,
    # so keep many rows.
    # qt0-2 exchange once per q-tile (fewer sync-queue wait points in the
    # steady state); the LAST q-tile splits per batch so the tail only
    # waits on a 64KB op and batch0's half hides under the p==1 pass
    cc_in = [
        [nc.dram_tensor(f"cc_in{q}", [N_CORES * 128, 128], dt.float8e4)]
        if q < NQT - 1
        else [
            nc.dram_tensor(f"cc_in{q}_{b}", [N_CORES * 128, 64], dt.float8e4)
            for b in range(2)
        ]
        for q in range(NQT)
    ]
    cc_out = [
        [nc.dram_tensor(f"cc_out{q}", [N_CORES * 128, 128], dt.float8e4)]
        if q < NQT - 1
        else [
            nc.dram_tensor(f"cc_out{q}_{b}", [N_CORES * 128, 64], dt.float8e4)
            for b in range(2)
        ]
        for q in range(NQT)
    ]
    # tiny dummy exchange fired at kernel start: absorbs the ~11.5us
    # first-collective trigger delay + CC DMA-ring spin-up so A2A(qt0)
    # runs at warm-stream speed
    cc_warm_in = nc.dram_tensor("cc_warm_in", [N_CORES, 128], dt.bfloat16)
    cc_warm_out = nc.dram_tensor("cc_warm_out", [N_CORES, 128], dt.bfloat16)

    with tile.TileContext(nc) as tc:
        with (
            tc.tile_pool(name="persist", bufs=1) as persist,
            tc.tile_pool(name="pT_pool", bufs=6) as pT_pool,
            tc.tile_pool(name="work", bufs=3) as work,
            tc.tile_pool(name="ln", bufs=2) as lnp,
        ):
            # ---------------- persistent SBUF loads ----------------
            zero_sb = persist.tile([128, 1], dt.float32, name="zero_sb")
            nc.vector.memset(zero_sb, 0.0)
            nc.const_aps.aps[(dt.float32, 0.0)] = zero_sb
            eps_sb = persist.tile([128, 1], dt.float32, name="eps_sb")
            nc.vector.memset(eps_sb, LN_EPS)
            # warm the CC stream before any data dep can delay the trigger
            nc.gpsimd.collective_compute(
                "AllToAll",
                Alu.bypass,
                replica_groups=REPLICA_GROUPS,
                ins=[cc_warm_in[:, :].opt()],
                outs=[cc_warm_out[:, :].opt()],
            )
            # input DMAs: interleaved so the c-outer QK matmuls can start
            # after the first hsT/wqk chunk pair lands (the sync queue
            # serializes at ~0.6us per dma_start dispatch, so keep them few).
            # wd/res (3MB) aren't consumed until ~120us in: dispatch them
            # last so they don't steal HBM bandwidth from the hsT stream.
            hsT_all = persist.tile([128, NCD, B * S], dt.float8e4, name="hsT_all")
            hsT_r = hsT[:, :].rearrange("(c p) t -> p c t", p=128)
            wqk_all = persist.tile([128, NCD, 256], dt.float8e4, name="wqk_all")
            wqk_r = wqk[:, :].rearrange("(c p) n -> p c n", p=128)
            nc.sync.dma_start(out=wqk_all[:, 0:8, :], in_=wqk_r[:, 0:8, :])
            nc.sync.dma_start(out=hsT_all[:, 0:2, :], in_=hsT_r[:, 0:2, :])
            nc.sync.dma_start(out=hsT_all[:, 2:4, :], in_=hsT_r[:, 2:4, :])
            nc.sync.dma_start(out=hsT_all[:, 4:6, :], in_=hsT_r[:, 4:6, :])
            nc.sync.dma_start(out=hsT_all[:, 6:8, :], in_=hsT_r[:, 6:8, :])
            bqk_all = persist.tile([128, 2], dt.float32, name="bqk_all")
            nc.sync.dma_start(
                out=bqk_all, in_=bqk[:, :].rearrange("(m p) o -> p (m o)", p=128)
            )
            wv_all = persist.tile([128, NCD, 128], dt.float8e4, name="wv_all")
            nc.sync.dma_start(
                out=wv_all, in_=wv[:, :].rearrange("(c p) n -> p c n", p=128)
            )
            wd_all = persist.tile([128, NCD, HIDDEN], dt.float8e4, name="wd_all")
            nc.sync.dma_start(
                out=wd_all, in_=wd[:, :].rearrange("(c p) n -> p c n", p=128)
            )
            res_all = persist.tile([128, NQT, HIDDEN], dt.bfloat16, name="res_all")
            nc.sync.dma_start(
                out=res_all,
                in_=hs_res[:, :].rearrange("(g p) n -> p g n", p=128),
            )
            bqk_sb = [bqk_all[:, m : m + 1] for m in range(2)]

            # qkT m-chunk layout: 0=K batch0, 1=Q batch0, 2=K batch1, 3=Q batch1
            # (partitions 0:64 = local head 0, 64:128 = local head 1)
            qkT_sb = [
                persist.tile([128, S], dt.bfloat16, name=f"qkT{m}") for m in range(4)
            ]
            # V tiles (fp8, DoubleRow pairs): tile t2 slot s covers token
            # chunk 2*t2+s as 4 groups (g = 2*batch + head) of [V_h(64) | ones(64)]
            v2_sb = [
                persist.tile([128, 2, 512], dt.float8e4, name=f"v{t2}")
                for t2 in range(NTOK // 2)
            ]
            # the denominator 'ones' (=32, matching the x32 wv prescale) never
            # change: write them all here while the vector engine is idle
            # instead of inside the qt0 attention weave
            for t2 in range(NTOK // 2):
                vt_all = v2_sb[t2].rearrange("p s (g c) -> p (s g) c", c=128)
                nc.vector.memset(vt_all[:, :, 64:128], 32.0)
            # ctx^T (normalized, bf16): chunk p = batch p, partitions 0:64 =
            # local head 0, 64:128 = local head 1, cols = batch p's tokens
            ctxT_sb = [
                persist.tile([128, S], dt.float8e4, name=f"ctxT{p}")
                for p in range(PAIRS)
            ]

            # ---------------- QK projection (c-outer, all 8 PSUM banks) -------
            # qk_ps region idx = dm*4 + nh*2 + j accumulates over c; iteration c
            # only needs hsT chunk c + wqk chunk c, so compute starts while the
            # rest of the inputs are still in flight. m-chunk m: batch m//2,
            # K/Q = m%2 (wqk cols (m%2)*128).
            with tc.tile_pool(name="psqk", bufs=1, space="PSUM") as psqk:
                for mp in range(2):  # m-pass: m in {2mp, 2mp+1} = batch mp
                    qk_ps = psqk.tile([128, 8, 512], dt.float32, name="qk_ps")
                    for cp in range(NCD // 2):
                        for dm in range(2):
                            m = 2 * mp + dm
                            for nh in range(2):
                                for j in range(2):
                                    nc.tensor.matmul(
                                        qk_ps[:, dm * 4 + nh * 2 + j, :],
                                        lhsT=wqk_all[
                                            :,
                                            2 * cp : 2 * cp + 2,
                                            dm * 128 : (dm + 1) * 128,
                                        ],
                                        rhs=hsT_all[
                                            :,
                                            2 * cp : 2 * cp + 2,
                                            mp * 2048
                                            + nh * 1024
                                            + j * 512 : mp * 2048
                                            + nh * 1024
                                            + (j + 1) * 512,
                                        ],
                                        start=(cp == 0),
                                        stop=(cp == NCD // 2 - 1),
                                        perf_mode=mybir.MatmulPerfMode.DoubleRow,
                                    )
                    for dm in range(2):
                        m = 2 * mp + dm
                        for nh in range(2):
                            nc.scalar.activation(
                                out=qkT_sb[m][:, nh * 1024 : (nh + 1) * 1024],
                                in_=qk_ps[
                                    :, dm * 4 + nh * 2 : dm * 4 + nh * 2 + 2, :
                                ],
                                func=Act.Identity,
                                bias=bqk_sb[dm],
                            )

            # psqk released; attention pools take over PSUM
            with (
                tc.tile_pool(name="psmm", bufs=3, space="PSUM") as psmm,
                tc.tile_pool(name="psctx", bufs=1, space="PSUM") as psctx,
            ):
                # V[tc] group g=2b+l: cols l*... ps[:, b*128+l*64 : +64] =
                # hs[b, tok_chunk] @ wv[:, l*64:...]; v tile cols g*128+64 :
                # (g+1)*128 are constant 32.0 (denominator trick)
                def emit_v_chunk(t):
                    ps = psmm.tile([128, 1024], dt.float32, name="ps_mm")
                    for b in range(2):
                        for cp in range(NCD // 2):
                            nc.tensor.matmul(
                                ps[:, b * 128 : (b + 1) * 128],
                                lhsT=hsT_all[
                                    :,
                                    2 * cp : 2 * cp + 2,
                                    b * 2048 + t * 128 : b * 2048 + (t + 1) * 128,
                                ],
                                rhs=wv_all[:, 2 * cp : 2 * cp + 2, :],
                                start=(cp == 0),
                                stop=(cp == NCD // 2 - 1),
                                perf_mode=mybir.MatmulPerfMode.DoubleRow,
                            )
                    vt = v2_sb[t // 2][:, t % 2, :].rearrange(
                        "p (g c) -> p g c", c=128
                    )
                    # alternate the evacuation engine so qt0's V weave doesn't
                    # pile 12 copies onto the vector engine alone
                    if t % 2 == 0:
                        nc.scalar.activation(
                            out=vt[:, :, 0:64],
                            in_=ps[:, 0:256].rearrange("p (g c) -> p g c", c=64),
                            func=Act.Identity,
                        )
                    else:
                        nc.vector.tensor_copy(
                            out=vt[:, :, 0:64],
                            in_=ps[:, 0:256].rearrange("p (g c) -> p g c", c=64),
                        )

                for t in range(4):
                    emit_v_chunk(t)

                # ------------- phase 2: attention + A2A + dense + LN ----------
                # q-tile-major; after each q-tile's ctx is normalized, the
                # chunk's ctxT slices are DMAed out and an 8-core AllToAll
                # fires. dense+LN for qt-1 are woven into qt's second (p==1)
                # attention pass, by which point A2A(qt-1) has long landed.
                def emit_a2a(qt, b):
                    # batch b's ctxT slice is final right after pass p=b's
                    # normalize: stage it immediately; trigger per-half for
                    # the last q-tile, once per q-tile otherwise
                    if qt < NQT - 1:
                        cin = cc_in[qt][0][:, :].rearrange(
                            "(r c) (bb t) -> c r bb t", c=128, t=64
                        )[:, :, b, :]
                    else:
                        cin = cc_in[qt][b][:, :].rearrange(
                            "(r c) t -> c r t", c=128
                        )
                    nc.sync.dma_start(
                        out=cin,
                        in_=ctxT_sb[b][
                            :, qt * 512 : (qt + 1) * 512
                        ].rearrange("c (r t) -> c r t", t=64),
                    )
                    if qt == NQT - 1 or b == 1:
                        idx = b if qt == NQT - 1 else 0
                        nc.gpsimd.collective_compute(
                            "AllToAll",
                            Alu.bypass,
                            replica_groups=REPLICA_GROUPS,
                            ins=[cc_in[qt][idx][:, :].opt()],
                            outs=[cc_out[qt][idx][:, :].opt()],
                        )

                # dense + residual + LN for one q-tile's 128-token shard,
                # staged so each piece slots into engine slack of the covering
                # attention pass (fetch / matmul blob+evac / stats / finish)
                dense_state = {}

                def emit_ctx_fetch(qt):
                    ctx_sb = work.tile([128, NCD, 128], dt.float8e4, name="ctx_sb")
                    if qt < NQT - 1:
                        nc.sync.dma_start(
                            out=ctx_sb,
                            in_=cc_out[qt][0][:, :].rearrange(
                                "(c p) t -> p c t", p=128
                            ),
                        )
                    else:
                        for b in range(2):
                            nc.sync.dma_start(
                                out=ctx_sb[:, :, b * 64 : (b + 1) * 64],
                                in_=cc_out[qt][b][:, :].rearrange(
                                    "(c p) t -> p c t", p=128
                                ),
                            )
                    dense_state["ctx_sb"] = ctx_sb

                def emit_dense(qt):
                    # 16 matmuls + immediate add-evacuation (x = dense + res).
                    # ps_d's full lifetime is inside this call, so sharing the
                    # ps_mm rotation with the scores pipeline is safe.
                    ctx_sb = dense_state["ctx_sb"]
                    ps_d = psmm.tile([128, 1024], dt.float32, name="ps_mm")
                    for cp in range(NCD // 2):
                        for j in range(2):
                            nc.tensor.matmul(
                                ps_d[:, j * 512 : (j + 1) * 512],
                                lhsT=ctx_sb[:, 2 * cp : 2 * cp + 2, :],
                                rhs=wd_all[:, 2 * cp : 2 * cp + 2, j * 512 : (j + 1) * 512],
                                start=(cp == 0),
                                stop=(cp == NCD // 2 - 1),
                                perf_mode=mybir.MatmulPerfMode.DoubleRow,
                            )
                    x = lnp.tile([128, HIDDEN], dt.float32, name="x")
                    nc.vector.tensor_tensor(
                        out=x, in0=ps_d, in1=res_all[:, qt, :], op=Alu.add
                    )
                    dense_state["x"] = x

                def emit_ln_stats(qt):
                    x = dense_state["x"]
                    stats = lnp.tile([128, 2, 6], dt.float32, name="stats")
                    xv = x.rearrange("p (s f) -> p s f", f=512)
                    for i in range(2):
                        nc.vector.bn_stats(out=stats[:, i, :], in_=xv[:, i, :])
                    mv = lnp.tile([128, 2], dt.float32, name="mv")
                    nc.vector.bn_aggr(out=mv, in_=stats)
                    dense_state["mv"] = mv

                def emit_ln_fin(qt):
                    x = dense_state["x"]
                    mv = dense_state["mv"]
                    # rstd = exp(-0.5 * ln(var + eps)) -- stays in the exp/ln tables
                    lnv = lnp.tile([128, 1], dt.float32, name="lnv")
                    nc.scalar.activation(
                        out=lnv, in_=mv[:, 1:2], func=Act.Ln, bias=eps_sb
                    )
                    rstd = lnp.tile([128, 1], dt.float32, name="rstd")
                    nc.scalar.activation(
                        out=rstd, in_=lnv, func=Act.Exp, scale=-0.5
                    )
                    y = lnp.tile([128, HIDDEN], dt.float32, name="y")
                    nc.vector.tensor_scalar(
                        out=y,
                        in0=x,
                        scalar1=mv[:, 0:1],
                        scalar2=rstd,
                        op0=Alu.subtract,
                        op1=Alu.mult,
                    )
                    nc.sync.dma_start(
                        out=out[qt * 128 : (qt + 1) * 128, :], in_=y
                    )

                WEAVE = {
                    (2, 0, 10): (emit_ctx_fetch, 0),
                    (2, 0, 14): (emit_dense, 0),
                    (2, 1, 4): (emit_ln_stats, 0),
                    (2, 1, 8): (emit_ln_fin, 0),
                    (2, 1, 14): (emit_ctx_fetch, 1),
                    (3, 0, 2): (emit_dense, 1),
                    (3, 0, 6): (emit_ln_stats, 1),
                    (3, 0, 10): (emit_ln_fin, 1),
                    (3, 1, 2): (emit_ctx_fetch, 2),
                    (3, 1, 6): (emit_dense, 2),
                    (3, 1, 10): (emit_ln_stats, 2),
                }

                for qt in range(NQT):
                    for p in range(PAIRS):
                        km = 2 * p  # K m-chunk (batch p)
                        qm = 2 * p + 1  # Q m-chunk (batch p)
                        ctx_ps = [
                            psctx.tile([128, 512], dt.float32, name=f"ps_ctx{l}")
                            for l in range(2)
                        ]

                        def emit_scores(kc, km=km, qm=qm, qt=qt):
                            ps_s = psmm.tile([128, 1024], dt.float32, name="ps_mm")
                            # scores^T for both local heads (concurrent row
                            # groups: head0 rows 0:64, head1 rows 64:128)
                            for l in range(2):
                                nc.tensor.matmul(
                                    ps_s[:, l * 512 : (l + 1) * 512],
                                    lhsT=qkT_sb[km][
                                        l * 64 : (l + 1) * 64, kc * 128 : (kc + 1) * 128
                                    ],
                                    rhs=qkT_sb[qm][
                                        l * 64 : (l + 1) * 64, qt * 512 : (qt + 1) * 512
                                    ],
                                    start=True,
                                    stop=True,
                                    tile_position=(l * 64, 0),
                                )
                            return ps_s

                        # software pipeline: scores run one k-chunk ahead so the
                        # PE never sits in-order behind ctx(k)'s wait on exp(k).
                        # probs are written as fp8 in kc pairs; each pair is one
                        # DoubleRow ctx matmul (2 fp8 weights/cell, K=256).
                        ps_s = emit_scores(0)
                        pT2 = None
                        for kc in range(NTOK):
                            kc2, sl = kc // 2, kc % 2
                            if sl == 0:
                                pT2 = pT_pool.tile(
                                    [128, 2, 1024], dt.float8e4, name="pT2"
                                )
                            ps_s_next = emit_scores(kc + 1) if kc + 1 < NTOK else None
                            if kc in DVE_KC:
                                # vector-engine poly exp (frees the ACT engine)
                                nc.vector._custom_dve(
                                    EXP_OP,
                                    out=pT2[:, sl, :],
                                    in0=ps_s,
                                    s0=EXP_S0,
                                    s1=EXP_S1,
                                    imm2=EXP_IMM2,
                                )
                            else:
                                nc.scalar.activation(
                                    out=pT2[:, sl, :],
                                    in_=ps_s,
                                    func=Act.Exp,
                                    scale=0.125 / 1024,
                                )
                            ps_s = ps_s_next
                            # ctx^T (+ denominator rows 64:128): one DoubleRow
                            # matmul per kc pair per head, accumulated over kc2
                            if sl == 1:
                                for l in range(2):
                                    g = 2 * p + l
                                    nc.tensor.matmul(
                                        ctx_ps[l],
                                        lhsT=v2_sb[kc2][
                                            :, :, g * 128 : (g + 1) * 128
                                        ],
                                        rhs=pT2[:, :, l * 512 : (l + 1) * 512],
                                        start=(kc2 == 0),
                                        stop=(kc2 == NTOK // 2 - 1),
                                        perf_mode=mybir.MatmulPerfMode.DoubleRow,
                                    )
                            # first q-tile: produce the remaining V chunks just
                            # ahead of their use (ctx(kc) needs v_sb[kc]); later
                            # q-tiles: weave previous q-tiles' dense+LN stages
                            # (which consume those q-tiles' A2As) per WEAVE.
                            # The pipeline runs ~1.5 q-tiles behind attention:
                            # the early collectives are 2-3x slower than steady
                            # state, and a fetch dispatched before its A2A
                            # completes would block the in-order sync queue
                            # (delaying the next q-tile's staging DMAs).
                            if p == 0 and qt == 0 and kc + 4 < NTOK:
                                emit_v_chunk(kc + 4)
                            act = WEAVE.get((qt, p, kc))
                            if act is not None:
                                fn, dqt = act
                                fn(dqt)
                        # normalize: ctx[0:64] / den[64:128] -> ctxT (fp8);
                        # both heads' denominators share one reciprocal pass
                        # (reciprocal_approx_fast must NOT read PSUM directly:
                        # that produced NaNs; the SBUF den2 copy is load-bearing)
                        den2 = work.tile([128, 512], dt.float32, name="den2")
                        for l in range(2):
                            nc.vector.tensor_copy(
                                out=den2[l * 64 : (l + 1) * 64, :],
                                in_=ctx_ps[l][64:128, :],
                            )
                        rec = work.tile([128, 512], dt.float32, name="rec")
                        nc.vector.reciprocal_approx_fast(out=rec, in_=den2)
                        for l in range(2):
                            nc.vector.tensor_tensor(
                                out=ctxT_sb[p][
                                    l * 64 : (l + 1) * 64, qt * 512 : (qt + 1) * 512
                                ],
                                in0=ctx_ps[l][0:64, :],
                                in1=rec[l * 64 : (l + 1) * 64, :],
                                op=Alu.mult,
                            )
                        emit_a2a(qt, p)
                # last q-tile's dense+LN have no following attention to hide
                # in; qt2's LN tail fills the final exchange's flight time
                emit_ln_fin(NQT - 2)
                emit_ctx_fetch(NQT - 1)
                emit_dense(NQT - 1)
                emit_ln_stats(NQT - 1)
                emit_ln_fin(NQT - 1)

    nc.compile()
    return nc


_PROGRAM = None


def _get_program():
    global _PROGRAM
    if _PROGRAM is None:
        _PROGRAM = _build_program()
    return _PROGRAM


def _prep_core_inputs(hidden_states, w_qkv, b_qkv, w_dense, b_dense):
    """Build the 8 per-core input maps (numpy, host-side sharding)."""
    hs = np.asarray(hidden_states, dtype=np.float32)
    w_qkv = np.asarray(w_qkv, dtype=np.float32)
    b_qkv = np.asarray(b_qkv, dtype=np.float32)
    w_dense = np.asarray(w_dense, dtype=np.float32)
    b_dense = np.asarray(b_dense, dtype=np.float32)

    # v-channel bias folded into a host-side output bias:
    # b_out = b_dense + b_v_full @ w_dense   (b_v in ctx channel order)
    bv_full = np.empty((HIDDEN,), dtype=np.float64)
    for g in range(HEADS):
        bv_full[g * HD : (g + 1) * HD] = b_qkv[g * 192 + 128 : g * 192 + 192]
    # w_dense rows are already in (head, d) = g*64+d order, matching bv_full
    b_out = (
        b_dense.astype(np.float64)
        + bv_full @ w_dense.astype(np.float64)
    ).astype(np.float32)

    # shared across cores: both batches' hs^T in fp8, full dense weight
    hsT_bf = np.concatenate(
        [np.ascontiguousarray(hs[0].T), np.ascontiguousarray(hs[1].T)], axis=1
    ).astype(FP8)  # [1024, 4096]
    # x256 prescale keeps wd in fp8e4 normal range; the dense partials come
    # out x256 and the residual is prescaled to match (LN is scale-invariant)
    wd_bf = (w_dense * 256).astype(FP8)  # [1024, 1024], rows channel-ordered

    in_maps = []
    for r in range(N_CORES):
        gheads = [2 * r, 2 * r + 1]

        # wqk column order: K h0 | K h1 | Q h0 | Q h1 (64 each)
        wqk_cols = np.empty((HIDDEN, 256), dtype=np.float32)
        bqk_vec = np.empty((256,), dtype=np.float32)
        for l, g in enumerate(gheads):
            kcol = slice(g * 192 + 64, g * 192 + 128)
            qcol = slice(g * 192, g * 192 + 64)
            wqk_cols[:, l * 64 : (l + 1) * 64] = w_qkv[:, kcol]
            wqk_cols[:, 128 + l * 64 : 128 + (l + 1) * 64] = w_qkv[:, qcol]
            bqk_vec[l * 64 : (l + 1) * 64] = b_qkv[kcol]
            bqk_vec[128 + l * 64 : 128 + (l + 1) * 64] = b_qkv[qcol]

        wv_cols = np.empty((HIDDEN, 128), dtype=np.float32)
        for l, g in enumerate(gheads):
            wv_cols[:, l * 64 : (l + 1) * 64] = w_qkv[
                :, g * 192 + 128 : g * 192 + 192
            ]

        # residual shard (+ folded output bias): row qt*128 + b*64 + t
        # covers full[b, qt*512 + r*64 + t]
        res = np.empty((OUT_ROWS, HIDDEN), dtype=np.float32)
        for qt in range(NQT):
            t0 = qt * 512 + r * 64
            for b in range(B):
                res[qt * 128 + b * 64 : qt * 128 + (b + 1) * 64, :] = 256 * (
                    hs[b, t0 : t0 + 64, :] + b_out
                )

        in_maps.append(
            {
                "hsT": hsT_bf,
                "wqk": (wqk_cols * 32).astype(FP8),
                "wv": (wv_cols * 32).astype(FP8),
                "wd": wd_bf,
                "bqk": (bqk_vec * 32).reshape(256, 1),
                "hs_res": res.astype(BF16),
            }
        )
    return in_maps


def kernel(hidden_states, w_qkv, b_qkv, w_dense, b_dense, ln_gamma, ln_beta,
           _return_perf=False, **run_kwargs):
    ln_gamma = np.asarray(ln_gamma, dtype=np.float32)
    ln_beta = np.asarray(ln_beta, dtype=np.float32)
    gamma_one = np.allclose(ln_gamma, 1.0)
    beta_zero = np.allclose(ln_beta, 0.0)

    nc = _get_program()
    in_maps = _prep_core_inputs(hidden_states, w_qkv, b_qkv, w_dense, b_dense)
    res = run_bass_kernel_spmd(
        nc, in_maps, core_ids=list(range(N_CORES)), **run_kwargs
    )

    full = np.empty((B, S, HIDDEN), dtype=np.float32)
    for r in range(N_CORES):
        o = res.results[r]["out"]
        for qt in range(NQT):
            t0 = qt * 512 + r * 64
            for b in range(B):
                full[b, t0 : t0 + 64, :] = o[
                    qt * 128 + b * 64 : qt * 128 + (b + 1) * 64, :
                ]

    if not (gamma_one and beta_zero):
        # spec fills gamma=ones, beta=zeros; fall back on host if they differ
        full = full * ln_gamma[None, None, :] + ln_beta[None, None, :]

    if _return_perf:
        return full, res
    return full
